# revision 29
# baseline (speedup 1.0000x reference)
"""Anchor target-assignment (IoU match + encode) on 8 TRN2 NeuronCores.

Self-contained: hardcodes shapes for the nn_Anchors problem
(B=4, N=64, input_size=512, A=195840).

v2 design ("t-major" anchor reorder):
  Anchors are processed in (level, gy, t-block, gx) order so that each
  128-anchor tile has gx on partitions and a constant anchor-type block.
  - x-overlap factors wx(gx,t,b,n) live in 19 precomputed X-table tiles
    [128, 256] (computed on device from box corners, one relu pipeline per
    X-tile instead of per anchor-tile).
  - wy(gy,t,b,n) * invS(lv,t,b,n) is host-computed per tile as 1..8 rows
    [nb, 256]; a tiny PE ones-block matmul broadcasts it to [128, 256].
  - q = wx * (wy*invS) is a per-box monotone transform of IoU
    (q = inter/(area_a+area_b)); argmax_n q == argmax_n IoU and IoU
    thresholds 0.5/0.4 map to q thresholds 1/3 and 2/7.
  - argmax via tensor_reduce(max) + max_index (exact first-occurrence).
  - gather of matched (cx, cy, log w, log h, label) via bf16 one-hot
    PE transpose + matmul; coords/log-sizes are split hi+lo in bf16 to
    keep f32-level accuracy.
  Levels 1-3 pad their t-axis to tile multiples; padded X rows are zero
  (q=0) and the host drops pad slots when stitching.
"""

import math
import os
import sys

import numpy as np

for _p in ("/opt/trn_rl_repo", "/root/.axon_site/_ro/trn_rl_repo"):
    if os.path.isdir(_p) and _p not in sys.path:
        sys.path.insert(0, _p)

# ----------------------------------------------------------------------------
# Problem constants
# ----------------------------------------------------------------------------
B = 4
N = 64
S = 512
ANCHOR_AREAS = [4 * 4, 16 * 16, 64 * 64, 128 * 128]
ASPECT_RATIOS = [1 / 2.0, 1 / 1.0, 2 / 1.0]
SCALE_RATIOS = [1.0, 2.0, 3 / 4.0]
NT = 9  # anchor types per cell
FMS = [128, 64, 32, 16]
LEVEL_ANCHORS = [fm * fm * NT for fm in FMS]  # 147456, 36864, 9216, 2304
A_TOTAL = sum(LEVEL_ANCHORS)  # 195840
P = 128
NC_COUNT = 8
FREE = B * N  # 256

# t-blocking per level: tiles are [NB t-variants x (128/NB) gx cells]
NB = [1, 2, 4, 8]  # t rows per tile (level i)
GXW = [128, 64, 32, 16]  # gx cells per t row (= fm)
TBLK = [9, 5, 3, 2]  # t-blocks per gy row (t padded to NB*TBLK)
LEVEL_TILES = [FMS[i] * TBLK[i] for i in range(4)]  # 1152, 320, 96, 32
CORE_LEVEL_TILES = [t // NC_COUNT for t in LEVEL_TILES]  # 144, 40, 12, 4
TILES_PER_CORE = sum(CORE_LEVEL_TILES)  # 200
AC = TILES_PER_CORE * P  # 25600 slots per core
GROUPS = 8
GTILES = TILES_PER_CORE // GROUPS  # 25
XTILES = TBLK  # X-table tiles per level: 9, 5, 3, 2 -> 19
WYROWS = [CORE_LEVEL_TILES[i] * NB[i] for i in range(4)]  # 144, 80, 48, 32

_prog_cache = {}


# ----------------------------------------------------------------------------
# Host-side anchor construction (mirrors reference.py exactly)
# ----------------------------------------------------------------------------
def _anchor_wh():
    wh = []
    for s in ANCHOR_AREAS:
        for ar in ASPECT_RATIOS:
            h = math.sqrt(s / ar)
            w = ar * h
            for sr in SCALE_RATIOS:
                wh.append([w * sr, h * sr])
    return np.asarray(wh, np.float32).reshape(len(ANCHOR_AREAS), NT, 2)


def build_anchor_boxes(input_size):
    wh = _anchor_wh()
    out = []
    for i in range(len(ANCHOR_AREAS)):
        fm = int(math.ceil(input_size / 2.0 ** (i + 2)))
        grid = input_size / fm
        centers = (np.arange(fm, dtype=np.float32) + 0.5) * grid
        gx, gy = np.meshgrid(centers, centers)
        xy = np.stack([gx, gy], axis=-1)
        xy = np.broadcast_to(xy[:, :, None, :], (fm, fm, NT, 2))
        whl = np.broadcast_to(wh[i][None, None, :, :], (fm, fm, NT, 2))
        out.append(
            np.concatenate([xy, whl], axis=-1).astype(np.float32).reshape(-1, 4)
        )
    return np.concatenate(out, 0)


def _slot_meta():
    """Per-slot (level, xtile_index, wyrow_start) — identical on all cores."""
    meta = []
    for lv in range(4):
        for j in range(CORE_LEVEL_TILES[lv]):
            xt = sum(XTILES[:lv]) + (j % TBLK[lv])
            wyr0 = sum(WYROWS[:lv]) + j * NB[lv]
            meta.append((lv, xt, wyr0))
    return meta


def _core_slot_anchor_index(core):
    """[TILES_PER_CORE, P] global real-anchor index per slot, or -1 for pads."""
    lvl_base = np.cumsum([0] + LEVEL_ANCHORS)
    idx = np.full((TILES_PER_CORE, P), -1, np.int64)
    s = 0
    for lv in range(4):
        nb, gxw, tb = NB[lv], GXW[lv], TBLK[lv]
        for j in range(CORE_LEVEL_TILES[lv]):
            gtile = core * CORE_LEVEL_TILES[lv] + j
            gy = gtile // tb
            tblk = gtile % tb
            p = np.arange(P)
            t = tblk * nb + p // gxw
            gx = p % gxw
            a = lvl_base[lv] + (gy * gxw + gx) * NT + t
            idx[s] = np.where(t < NT, a, -1)
            s += 1
    return idx


# ----------------------------------------------------------------------------
# Bass program (one SPMD program, identical for all 8 cores)
# ----------------------------------------------------------------------------
def _build_program():
    from contextlib import ExitStack

    from concourse import bacc, mybir
    from concourse.tile import TileContext

    fp32 = mybir.dt.float32
    bf16 = mybir.dt.bfloat16
    u32 = mybir.dt.uint32
    Alu = mybir.AluOpType
    Act = mybir.ActivationFunctionType

    nc = bacc.Bacc(None, target_bir_lowering=False)

    NX = sum(XTILES)  # 19

    # inputs
    corners_d = nc.declare_dram_parameter("corners", [P, 2 * FREE], fp32, isOutput=False)
    xclamp_d = nc.declare_dram_parameter("xclamp", [P, NX * 3], fp32, isOutput=False)
    encc_d = nc.declare_dram_parameter("encc", [P, TILES_PER_CORE * 6], fp32, isOutput=False)
    wyr_d = nc.declare_dram_parameter("wyrows", [88, 67 * FREE], bf16, isOutput=False)
    ones_d = nc.declare_dram_parameter("onesel", [P, 4 * P], bf16, isOutput=False)
    tables_d = nc.declare_dram_parameter("tables", [P, 36], bf16, isOutput=False)
    iota_d = nc.declare_dram_parameter("iota", [P, N], u32, isOutput=False)
    ident_d = nc.declare_dram_parameter("ident", [P, P], bf16, isOutput=False)
    # outputs (v-split loc + float cls), SBUF-natural order (host unpermutes)
    loc_d = [
        nc.declare_dram_parameter(f"loc{v}", [B, AC], fp32, isOutput=True)
        for v in range(4)
    ]
    cls_d = nc.declare_dram_parameter("clsf", [B, AC], fp32, isOutput=True)

    meta = _slot_meta()

    with TileContext(nc) as tc, ExitStack() as ctx:
        cpool = ctx.enter_context(tc.tile_pool(name="consts", bufs=1))
        corners = cpool.tile([P, 2, B, N], fp32)
        xclamp = cpool.tile([P, NX, 3], fp32)
        wyrows = cpool.tile([88, 67, FREE], bf16)
        onesel = cpool.tile([P, 4, P], bf16)
        tables = cpool.tile([P, 36], bf16)
        iota = cpool.tile([P, N], u32)
        ident = cpool.tile([P, P], bf16)
        nc.sync.dma_start(out=corners, in_=corners_d[:, :])
        nc.sync.dma_start(out=xclamp, in_=xclamp_d[:, :])
        nc.sync.dma_start(out=wyrows, in_=wyr_d[:, :])
        nc.sync.dma_start(out=onesel, in_=ones_d[:, :])
        nc.sync.dma_start(out=tables, in_=tables_d[:, :])
        nc.sync.dma_start(out=iota, in_=iota_d[:, :])
        nc.sync.dma_start(out=ident, in_=ident_d[:, :])

        xpool = ctx.enter_context(tc.tile_pool(name="xtab", bufs=1))
        xprep = ctx.enter_context(tc.tile_pool(name="xprep", bufs=3))
        gpool = ctx.enter_context(tc.tile_pool(name="group", bufs=2))
        wpool = ctx.enter_context(tc.tile_pool(name="work", bufs=8))
        spool = ctx.enter_context(tc.tile_pool(name="small", bufs=8))
        ppool = ctx.enter_context(tc.tile_pool(name="psum", bufs=2, space="PSUM"))
        gapool = ctx.enter_context(tc.tile_pool(name="gacc", bufs=2, space="PSUM"))
        stpool = ctx.enter_context(tc.tile_pool(name="stage", bufs=2))
        opool = ctx.enter_context(tc.tile_pool(name="outstage", bufs=1))

        # ---------------- X-table prep: 19 tiles ----------------
        xtab = [
            xpool.tile([P, B, N], fp32, tag=f"x{i}", name=f"x{i}") for i in range(NX)
        ]
        for i in range(NX):
            nax1 = xclamp[:, i : i + 1, 0:1]
            ax2 = xclamp[:, i : i + 1, 1:2]
            wa1p = xclamp[:, i : i + 1, 2:3]
            rsx = xprep.tile([P, B, N], fp32, tag="rsx")
            rtx = xprep.tile([P, B, N], fp32, tag="rtx")
            nc.scalar.activation(rsx, corners[:, 0], Act.Relu, bias=nax1, scale=1.0)
            nc.scalar.activation(rtx, corners[:, 1], Act.Relu, bias=ax2, scale=-1.0)
            ssx = xprep.tile([P, B, N], fp32, tag="ssx")
            nc.vector.scalar_tensor_tensor(
                ssx, rsx, wa1p, rtx, op0=Alu.subtract, op1=Alu.add
            )
            nc.scalar.activation(xtab[i], ssx, Act.Relu, bias=0.0, scale=-1.0)

        st = [
            opool.tile([P, B, TILES_PER_CORE], fp32, tag=f"st{v}", name=f"st{v}")
            for v in range(4)
        ]
        cls_st = opool.tile([P, B, TILES_PER_CORE], fp32, tag="stc")

        for g in range(GROUPS):
            encc_g = gpool.tile([P, GTILES, 6], fp32, tag="encc")
            nc.sync.dma_start(
                out=encc_g, in_=encc_d[:, g * GTILES * 6 : (g + 1) * GTILES * 6]
            )
            m_acc = gpool.tile([P, GTILES, B, 8], fp32, tag="macc")
            nc.vector.memset(m_acc, -1.0)

            g01_ps = gapool.tile([P, GTILES, 18], fp32, tag="g01")
            g23_ps = gapool.tile([P, GTILES, 18], fp32, tag="g23")

            for gi in range(GTILES):
                slot = g * GTILES + gi
                lv, xt, wyr0 = meta[slot]
                nb = NB[lv]

                # --- wyrep: 3-term bf16 broadcast matmul (sum is ~exact f32) ---
                wyrep_ps = ppool.tile([P, B, N], fp32, tag="wyrep")
                pb = 32 * (slot % 3)
                nc.tensor.matmul(
                    wyrep_ps,
                    onesel[pb : pb + 3 * nb, lv, :],
                    wyrows[pb : pb + 3 * nb, slot // 3, :],
                    start=True,
                    stop=True,
                )
                q = wpool.tile([P, B, N], fp32, tag="q")
                nc.vector.tensor_mul(q, xtab[xt], wyrep_ps)

                # --- max over boxes + argmax + one-hot ---
                nc.vector.tensor_reduce(
                    out=m_acc[:, gi : gi + 1, :, 0:1],
                    in_=q,
                    axis=mybir.AxisListType.X,
                    op=Alu.max,
                )
                i8 = spool.tile([P, B, 8], u32, tag="i8")
                for b in range(B):
                    nc.vector.max_index(i8[:, b, :], m_acc[:, gi, b, :], q[:, b, :])
                # one-hot in a single broadcast TT (uint32 compare, no cast)
                onehot = wpool.tile([P, B, N], bf16, tag="onehot")
                nc.vector.tensor_tensor(
                    out=onehot,
                    in0=iota.unsqueeze(1).to_broadcast([P, B, N]),
                    in1=i8[:, :, 0:1].to_broadcast([P, B, N]),
                    op=Alu.is_equal,
                )

                # --- gather via PE: transpose one-hot, matmul with tables ---
                ohT_ps = ppool.tile([P, 2, P], bf16, tag="ohT")
                oh2 = onehot.rearrange("p b n -> p (b n)")
                nc.tensor.transpose(ohT_ps[:, 0, :], oh2[:, 0:P], ident)
                nc.tensor.transpose(ohT_ps[:, 1, :], oh2[:, P : 2 * P], ident)
                ohT = spool.tile([P, 2, P], bf16, tag="ohTs")
                nc.scalar.copy(ohT, ohT_ps)
                nc.tensor.matmul(
                    g01_ps[:, gi, :], ohT[:, 0, :], tables[:, 0:18],
                    start=True, stop=True,
                )
                nc.tensor.matmul(
                    g23_ps[:, gi, :], ohT[:, 1, :], tables[:, 18:36],
                    start=True, stop=True,
                )

            # ---------------- per-group encode ----------------
            # gathered cols per (pair j): j*9 + [cxh,cxl,cyh,cyl,lwh,lwl,lhh,lhl,lab]
            g01 = stpool.tile([P, GTILES, 18], fp32, tag="g01s")
            g23 = stpool.tile([P, GTILES, 18], fp32, tag="g23s")
            nc.scalar.copy(g01, g01_ps)
            nc.scalar.copy(g23, g23_ps)

            gsl = slice(g * GTILES, (g + 1) * GTILES)
            enc = encc_g  # [P, GTILES, 6]: acx, acy, invaw, invah, lwa, lha

            for bp, gsb in ((0, g01), (1, g23)):
                gq = gsb.rearrange("p t (j f) -> p t j f", j=2)  # [P,GT,2,9]

                def outv(tile):
                    return tile[:, 2 * bp : 2 * bp + 2, gsl].transpose([0, 2, 1])

                # center coords: (hi + lo - a) * inv_size
                for v, si in ((0, 2), (1, 3)):
                    hl = stpool.tile([P, GTILES, 2], fp32, tag="enchl")
                    nc.vector.tensor_add(
                        hl, gq[:, :, :, 2 * v], gq[:, :, :, 2 * v + 1]
                    )
                    d = stpool.tile([P, GTILES, 2], fp32, tag="encd")
                    nc.vector.tensor_sub(
                        d, hl, enc[:, :, v : v + 1].to_broadcast([P, GTILES, 2])
                    )
                    nc.vector.tensor_mul(
                        outv(st[v]),
                        d,
                        enc[:, :, si : si + 1].to_broadcast([P, GTILES, 2]),
                    )
                # log-space wh: (lwh + lwl) - log(aw)
                for v, li in ((2, 4), (3, 5)):
                    hl = stpool.tile([P, GTILES, 2], fp32, tag="enchl")
                    nc.vector.tensor_add(
                        hl, gq[:, :, :, 2 * v], gq[:, :, :, 2 * v + 1]
                    )
                    nc.vector.tensor_sub(
                        outv(st[v]),
                        hl,
                        enc[:, :, li : li + 1].to_broadcast([P, GTILES, 2]),
                    )

            # cls: lab*a - (b2 - a);  a = [q >= 1/3], b2 = [q > 2/7]
            mvals = m_acc[:, :, :, 0]  # [P, GTILES, B] stride 8
            amask = stpool.tile([P, GTILES, B], fp32, tag="amask")
            bmask = stpool.tile([P, GTILES, B], fp32, tag="bmask")
            nc.vector.tensor_scalar(
                amask, mvals, float(np.float32(1.0) / np.float32(3.0)), None,
                op0=Alu.is_ge,
            )
            nc.vector.tensor_scalar(
                bmask, mvals, float(np.float32(0.4) / np.float32(1.4)), None,
                op0=Alu.is_gt,
            )
            ba = stpool.tile([P, GTILES, B], fp32, tag="ba")
            nc.vector.tensor_sub(ba, bmask, amask)
            for bp, gsb in ((0, g01), (1, g23)):
                lab = gsb.rearrange("p t (j f) -> p t j f", j=2)[:, :, :, 8]
                la = stpool.tile([P, GTILES, 2], fp32, tag="la")
                nc.vector.tensor_mul(la, lab, amask[:, :, 2 * bp : 2 * bp + 2])
                nc.vector.tensor_sub(
                    cls_st[:, 2 * bp : 2 * bp + 2, gsl].transpose([0, 2, 1]),
                    la,
                    ba[:, :, 2 * bp : 2 * bp + 2],
                )

        # outputs in SBUF-natural order: DRAM[b, p*T + ti] (host unpermutes)
        for v in range(4):
            for b in range(B):
                nc.sync.dma_start(
                    out=loc_d[v][b].rearrange("(p t) -> p t", p=P),
                    in_=st[v][:, b, :],
                )
        for b in range(B):
            nc.sync.dma_start(
                out=cls_d[b].rearrange("(p t) -> p t", p=P),
                in_=cls_st[:, b, :],
            )

    nc.compile()
    return nc


# ----------------------------------------------------------------------------
# Host data preparation
# ----------------------------------------------------------------------------
def _hilo(v):
    import ml_dtypes

    hi = v.astype(ml_dtypes.bfloat16)
    lo = (v - hi.astype(np.float32)).astype(ml_dtypes.bfloat16)
    return hi.astype(np.float32), lo.astype(np.float32)


def _prepare_host(labels, boxes):
    import ml_dtypes

    f32 = np.float32
    bfl = ml_dtypes.bfloat16
    NX = sum(XTILES)

    wh_t = _anchor_wh()  # [4, NT, 2]
    aa_lvl = (wh_t[..., 0] + f32(1.0)) * (wh_t[..., 1] + f32(1.0))  # [4, NT]

    a_, b_ = boxes[..., :2].astype(f32), boxes[..., 2:].astype(f32)
    bxy = (a_ + b_) / f32(2.0)
    bwh = b_ - a_ + f32(1.0)
    b1 = np.concatenate([bxy - bwh / f32(2.0), bxy + bwh / f32(2.0)], -1)  # [B,N,4]
    area_b = (b1[..., 2] - b1[..., 0] + f32(1.0)) * (
        b1[..., 3] - b1[..., 1] + f32(1.0)
    )

    corners = np.empty((P, 2, B, N), f32)
    corners[:, 0] = b1[None, :, :, 0]  # bx1
    corners[:, 1] = b1[None, :, :, 2]  # bx2

    grids = []
    for i in range(4):
        fm = FMS[i]
        grid = S / fm
        grids.append(((np.arange(fm, dtype=f32) + f32(0.5)) * f32(grid)).astype(f32))

    # X-clamp constants per X-tile; pad rows force wx = 0
    xclamp = np.zeros((P, NX, 3), f32)
    xi = 0
    for lv in range(4):
        nb, gxw = NB[lv], GXW[lv]
        for tb in range(TBLK[lv]):
            p = np.arange(P)
            t = tb * nb + p // gxw
            gx = p % gxw
            valid = t < NT
            tcl = np.clip(t, 0, NT - 1)
            w = wh_t[lv, tcl, 0]
            cx = grids[lv][gx]
            ax1 = cx - w / f32(2.0)
            ax2 = cx + w / f32(2.0)
            wa1p = ax2 - ax1 + f32(1.0)
            xclamp[:, xi, 0] = np.where(valid, -ax1, f32(0.0))
            xclamp[:, xi, 1] = np.where(valid, ax2, f32(-1.0e6))
            xclamp[:, xi, 2] = np.where(valid, wa1p, f32(-1.0))
            xi += 1

    # onesel[base + i, lv, p] = 1 where p//gxw == i % nb, for i < 3*nb
    onesel = np.zeros((P, 4, P), f32)
    for base in (0, 32, 64):
        for lv in range(4):
            nb, gxw = NB[lv], GXW[lv]
            for i in range(3 * nb):
                onesel[base + i, lv, (i % nb) * gxw : (i % nb + 1) * gxw] = 1.0

    # gather tables (bf16 hi/lo): [P, 36]
    lw = np.log(bwh[..., 0])
    lh = np.log(bwh[..., 1])
    labf = labels.astype(f32)
    tabs = np.zeros((P, 36), f32)
    for bp in range(2):
        for j in range(2):
            b = 2 * bp + j
            rows = slice(j * N, (j + 1) * N)
            cols = []
            for v in (bxy[b, :, 0], bxy[b, :, 1], lw[b], lh[b]):
                hi, lo = _hilo(v)
                cols += [hi, lo]
            cols.append(labf[b])
            tabs[rows, bp * 18 + j * 9 : bp * 18 + j * 9 + 9] = np.stack(cols, -1)
    tables = tabs.astype(bfl)

    iota = np.broadcast_to(np.arange(N, dtype=np.uint32)[None, :], (P, N)).copy()
    ident = np.eye(P, dtype=f32).astype(bfl)

    anchors = build_anchor_boxes(S)
    acx, acy, aw, ah = anchors[:, 0], anchors[:, 1], anchors[:, 2], anchors[:, 3]
    law, lah = np.log(aw), np.log(ah)
    invaw, invah = f32(1.0) / aw, f32(1.0) / ah

    percore = []
    for c in range(NC_COUNT):
        # wyrows = wy(gy,t) * invS(lv,t) per (b,n); [8, tile, 256] packing
        wyr = np.zeros((8, TILES_PER_CORE, FREE), f32)
        s = 0
        for lv in range(4):
            nb = NB[lv]
            cyv = grids[lv]
            for j in range(CORE_LEVEL_TILES[lv]):
                gtile = c * CORE_LEVEL_TILES[lv] + j
                gy = gtile // TBLK[lv]
                tblk = gtile % TBLK[lv]
                for k in range(nb):
                    t = tblk * nb + k
                    if t < NT:
                        h = wh_t[lv, t, 1]
                        cy = cyv[gy]
                        ay1 = cy - h / f32(2.0)
                        ay2 = cy + h / f32(2.0)
                        ha1p = ay2 - ay1 + f32(1.0)
                        rs = np.maximum(b1[..., 1] - ay1, f32(0.0))
                        rt = np.maximum(ay2 - b1[..., 3], f32(0.0))
                        wy = np.maximum(-((rs - ha1p) + rt), f32(0.0))
                        invs = f32(1.0) / (aa_lvl[lv, t] + area_b)
                        wyr[k, s] = (wy * invs).reshape(-1)
                s += 1
        # 3-term bf16 split packed per tile at partition base 32*(j%4)
        wyp = np.zeros((88, 67, FREE), f32)
        for j in range(TILES_PER_CORE):
            lv = next(
                i for i in range(4)
                if j < sum(CORE_LEVEL_TILES[: i + 1])
            )
            nb = NB[lv]
            base = 32 * (j % 3)
            col = j // 3
            for k in range(nb):
                w = wyr[k, j]
                hi = w.astype(bfl).astype(f32)
                mid = (w - hi).astype(bfl).astype(f32)
                lo = (w - hi - mid).astype(bfl).astype(f32)
                wyp[base + k, col] = hi
                wyp[base + nb + k, col] = mid
                wyp[base + 2 * nb + k, col] = lo
        wyrows = wyp.astype(bfl).reshape(88, 67 * FREE)

        gidx = _core_slot_anchor_index(c)  # [T, P], -1 = pad
        safe = np.where(gidx >= 0, gidx, 0)
        encc = np.stack(
            [acx[safe], acy[safe], invaw[safe], invah[safe], law[safe], lah[safe]],
            -1,
        ).astype(f32)
        encc = np.ascontiguousarray(encc.transpose(1, 0, 2)).reshape(
            P, TILES_PER_CORE * 6
        )

        percore.append(
            dict(
                corners=corners.reshape(P, 2 * FREE),
                xclamp=xclamp.reshape(P, NX * 3),
                encc=encc,
                wyrows=wyrows,
                onesel=onesel.astype(bfl).reshape(P, 4 * P),
                tables=tables,
                iota=iota,
                ident=ident,
            )
        )
    return percore


def _assemble(results, labels_dtype):
    cls_full = np.empty((B, A_TOTAL), np.float32)
    loc_full = np.empty((B, A_TOTAL, 4), np.float32)

    def unperm(a):
        # device emits [B, p*T + ti]; bring to slot-major [B, ti*P + p]
        return np.ascontiguousarray(
            a.reshape(B, P, TILES_PER_CORE).transpose(0, 2, 1).reshape(B, AC)
        )

    for c in range(NC_COUNT):
        r = results[c]
        gidx = _core_slot_anchor_index(c).reshape(-1)
        valid = gidx >= 0
        tgt = gidx[valid]
        cls_full[:, tgt] = unperm(r["clsf"])[:, valid]
        for v in range(4):
            loc_full[:, tgt, v] = unperm(r[f"loc{v}"])[:, valid]
    cls_out = cls_full.astype(labels_dtype)
    return cls_out, loc_full


def _fix_compiler_flags():
    """Skip the neuronxcc DataLocalityOpt pass (crashes on our DMA macros)."""
    from concourse import compiler_utils as cu

    flags = cu.get_compiler_flags()
    out = []
    for f in flags:
        if f.startswith("--tensorizer-options=") and "DataLocalityOpt" not in f:
            f = f.rstrip() + " --skip-pass=DataLocalityOpt "
        out.append(f)
    cu.set_compiler_flags(out)


def _run(labels, boxes, input_size, trace=False):
    from concourse.bass_utils import run_bass_kernel_spmd

    _fix_compiler_flags()

    assert int(input_size) == S, f"kernel hardcoded for input_size={S}"
    labels = np.asarray(labels)
    boxes = np.asarray(boxes, dtype=np.float32)

    if "prog" not in _prog_cache:
        _prog_cache["prog"] = _build_program()
    nc = _prog_cache["prog"]

    in_maps = _prepare_host(labels, boxes)
    res = run_bass_kernel_spmd(
        nc, in_maps, core_ids=list(range(NC_COUNT)), trace=trace
    )
    cls_out, loc_out = _assemble(res.results, labels.dtype)
    return (cls_out, loc_out), res


def kernel(labels, boxes, input_size):
    (cls_out, loc_out), _ = _run(labels, boxes, input_size)
    return cls_out, loc_out


# revision 30
# speedup vs baseline: 1.1541x; 1.1541x over previous
"""Anchor target-assignment (IoU match + encode) on 8 TRN2 NeuronCores.

Self-contained: hardcodes shapes for the nn_Anchors problem
(B=4, N=64, input_size=512, A=195840).

v2 design ("t-major" anchor reorder):
  Anchors are processed in (level, gy, t-block, gx) order so that each
  128-anchor tile has gx on partitions and a constant anchor-type block.
  - x-overlap factors wx(gx,t,b,n) live in 19 precomputed X-table tiles
    [128, 256] (computed on device from box corners, one relu pipeline per
    X-tile instead of per anchor-tile).
  - wy(gy,t,b,n) * invS(lv,t,b,n) is host-computed per tile as 1..8 rows
    [nb, 256]; a tiny PE ones-block matmul broadcasts it to [128, 256].
  - q = wx * (wy*invS) is a per-box monotone transform of IoU
    (q = inter/(area_a+area_b)); argmax_n q == argmax_n IoU and IoU
    thresholds 0.5/0.4 map to q thresholds 1/3 and 2/7.
  - argmax via tensor_reduce(max) + max_index (exact first-occurrence).
  - gather of matched (cx, cy, log w, log h, label) via bf16 one-hot
    PE transpose + matmul; coords/log-sizes are split hi+lo in bf16 to
    keep f32-level accuracy.
  Levels 1-3 pad their t-axis to tile multiples; padded X rows are zero
  (q=0) and the host drops pad slots when stitching.
"""

import math
import os
import sys

import numpy as np

for _p in ("/opt/trn_rl_repo", "/root/.axon_site/_ro/trn_rl_repo"):
    if os.path.isdir(_p) and _p not in sys.path:
        sys.path.insert(0, _p)

# ----------------------------------------------------------------------------
# Problem constants
# ----------------------------------------------------------------------------
B = 4
N = 64
S = 512
ANCHOR_AREAS = [4 * 4, 16 * 16, 64 * 64, 128 * 128]
ASPECT_RATIOS = [1 / 2.0, 1 / 1.0, 2 / 1.0]
SCALE_RATIOS = [1.0, 2.0, 3 / 4.0]
NT = 9  # anchor types per cell
FMS = [128, 64, 32, 16]
LEVEL_ANCHORS = [fm * fm * NT for fm in FMS]  # 147456, 36864, 9216, 2304
A_TOTAL = sum(LEVEL_ANCHORS)  # 195840
P = 128
NC_COUNT = 8
FREE = B * N  # 256

# t-blocking per level: tiles are [NB t-variants x (128/NB) gx cells]
NB = [1, 2, 4, 8]  # t rows per tile (level i)
GXW = [128, 64, 32, 16]  # gx cells per t row (= fm)
TBLK = [9, 5, 3, 2]  # t-blocks per gy row (t padded to NB*TBLK)
LEVEL_TILES = [FMS[i] * TBLK[i] for i in range(4)]  # 1152, 320, 96, 32
CORE_LEVEL_TILES = [t // NC_COUNT for t in LEVEL_TILES]  # 144, 40, 12, 4
TILES_PER_CORE = sum(CORE_LEVEL_TILES)  # 200
AC = TILES_PER_CORE * P  # 25600 slots per core
GROUPS = 8
GTILES = TILES_PER_CORE // GROUPS  # 25
XTILES = TBLK  # X-table tiles per level: 9, 5, 3, 2 -> 19
WYROWS = [CORE_LEVEL_TILES[i] * NB[i] for i in range(4)]  # 144, 80, 48, 32

_prog_cache = {}


# ----------------------------------------------------------------------------
# Host-side anchor construction (mirrors reference.py exactly)
# ----------------------------------------------------------------------------
def _anchor_wh():
    wh = []
    for s in ANCHOR_AREAS:
        for ar in ASPECT_RATIOS:
            h = math.sqrt(s / ar)
            w = ar * h
            for sr in SCALE_RATIOS:
                wh.append([w * sr, h * sr])
    return np.asarray(wh, np.float32).reshape(len(ANCHOR_AREAS), NT, 2)


def build_anchor_boxes(input_size):
    wh = _anchor_wh()
    out = []
    for i in range(len(ANCHOR_AREAS)):
        fm = int(math.ceil(input_size / 2.0 ** (i + 2)))
        grid = input_size / fm
        centers = (np.arange(fm, dtype=np.float32) + 0.5) * grid
        gx, gy = np.meshgrid(centers, centers)
        xy = np.stack([gx, gy], axis=-1)
        xy = np.broadcast_to(xy[:, :, None, :], (fm, fm, NT, 2))
        whl = np.broadcast_to(wh[i][None, None, :, :], (fm, fm, NT, 2))
        out.append(
            np.concatenate([xy, whl], axis=-1).astype(np.float32).reshape(-1, 4)
        )
    return np.concatenate(out, 0)


def _slot_meta():
    """Per-slot (level, xtile_index, wyrow_start) — identical on all cores."""
    meta = []
    for lv in range(4):
        for j in range(CORE_LEVEL_TILES[lv]):
            xt = sum(XTILES[:lv]) + (j % TBLK[lv])
            wyr0 = sum(WYROWS[:lv]) + j * NB[lv]
            meta.append((lv, xt, wyr0))
    return meta


def _core_slot_anchor_index(core):
    """[TILES_PER_CORE, P] global real-anchor index per slot, or -1 for pads."""
    lvl_base = np.cumsum([0] + LEVEL_ANCHORS)
    idx = np.full((TILES_PER_CORE, P), -1, np.int64)
    s = 0
    for lv in range(4):
        nb, gxw, tb = NB[lv], GXW[lv], TBLK[lv]
        for j in range(CORE_LEVEL_TILES[lv]):
            gtile = core * CORE_LEVEL_TILES[lv] + j
            gy = gtile // tb
            tblk = gtile % tb
            p = np.arange(P)
            t = tblk * nb + p // gxw
            gx = p % gxw
            a = lvl_base[lv] + (gy * gxw + gx) * NT + t
            idx[s] = np.where(t < NT, a, -1)
            s += 1
    return idx


# ----------------------------------------------------------------------------
# Bass program (one SPMD program, identical for all 8 cores)
# ----------------------------------------------------------------------------
def _build_program():
    from contextlib import ExitStack

    from concourse import bacc, mybir
    from concourse.tile import TileContext

    fp32 = mybir.dt.float32
    bf16 = mybir.dt.bfloat16
    u32 = mybir.dt.uint32
    Alu = mybir.AluOpType
    Act = mybir.ActivationFunctionType

    nc = bacc.Bacc(None, target_bir_lowering=False)

    NX = sum(XTILES)  # 19

    # inputs
    corners_d = nc.declare_dram_parameter("corners", [P, 2 * FREE], fp32, isOutput=False)
    xclamp_d = nc.declare_dram_parameter("xclamp", [P, NX * 3], fp32, isOutput=False)
    encc_d = nc.declare_dram_parameter("encc", [P, TILES_PER_CORE * 6], fp32, isOutput=False)
    wyr_d = nc.declare_dram_parameter("wyrows", [88, 67 * FREE], bf16, isOutput=False)
    ones_d = nc.declare_dram_parameter("onesel", [P, 4 * P], bf16, isOutput=False)
    tables_d = nc.declare_dram_parameter("tables", [P, 36], bf16, isOutput=False)
    iota_d = nc.declare_dram_parameter("iota", [P, N], fp32, isOutput=False)
    ident_d = nc.declare_dram_parameter("ident", [P, P], bf16, isOutput=False)
    # outputs (v-split loc + float cls), SBUF-natural order (host unpermutes)
    loc_d = [
        nc.declare_dram_parameter(f"loc{v}", [B, AC], fp32, isOutput=True)
        for v in range(4)
    ]
    cls_d = nc.declare_dram_parameter("clsf", [B, AC], fp32, isOutput=True)

    meta = _slot_meta()

    with TileContext(nc) as tc, ExitStack() as ctx:
        cpool = ctx.enter_context(tc.tile_pool(name="consts", bufs=1))
        corners = cpool.tile([P, 2, B, N], fp32)
        xclamp = cpool.tile([P, NX, 3], fp32)
        wyrows = cpool.tile([88, 67, FREE], bf16)
        onesel = cpool.tile([P, 4, P], bf16)
        tables = cpool.tile([P, 36], bf16)
        iota = cpool.tile([P, N], fp32)
        ident = cpool.tile([P, P], bf16)
        nc.sync.dma_start(out=corners, in_=corners_d[:, :])
        nc.sync.dma_start(out=xclamp, in_=xclamp_d[:, :])
        nc.sync.dma_start(out=wyrows, in_=wyr_d[:, :])
        nc.sync.dma_start(out=onesel, in_=ones_d[:, :])
        nc.sync.dma_start(out=tables, in_=tables_d[:, :])
        nc.sync.dma_start(out=iota, in_=iota_d[:, :])
        nc.sync.dma_start(out=ident, in_=ident_d[:, :])

        xpool = ctx.enter_context(tc.tile_pool(name="xtab", bufs=1))
        xprep = ctx.enter_context(tc.tile_pool(name="xprep", bufs=3))
        gpool = ctx.enter_context(tc.tile_pool(name="group", bufs=2))
        wpool = ctx.enter_context(tc.tile_pool(name="work", bufs=6))
        spool = ctx.enter_context(tc.tile_pool(name="small", bufs=6))
        ppool = ctx.enter_context(tc.tile_pool(name="psum", bufs=2, space="PSUM"))
        gapool = ctx.enter_context(tc.tile_pool(name="gacc", bufs=2, space="PSUM"))
        stpool = ctx.enter_context(tc.tile_pool(name="stage", bufs=2))
        opool = ctx.enter_context(tc.tile_pool(name="outstage", bufs=1))

        # ---------------- X-table prep: 19 tiles ----------------
        xtab = [
            xpool.tile([P, B, N], fp32, tag=f"x{i}", name=f"x{i}") for i in range(NX)
        ]
        for i in range(NX):
            nax1 = xclamp[:, i : i + 1, 0:1]
            ax2 = xclamp[:, i : i + 1, 1:2]
            wa1p = xclamp[:, i : i + 1, 2:3]
            rsx = xprep.tile([P, B, N], fp32, tag="rsx")
            rtx = xprep.tile([P, B, N], fp32, tag="rtx")
            nc.scalar.activation(rsx, corners[:, 0], Act.Relu, bias=nax1, scale=1.0)
            nc.scalar.activation(rtx, corners[:, 1], Act.Relu, bias=ax2, scale=-1.0)
            ssx = xprep.tile([P, B, N], fp32, tag="ssx")
            nc.vector.scalar_tensor_tensor(
                ssx, rsx, wa1p, rtx, op0=Alu.subtract, op1=Alu.add
            )
            nc.scalar.activation(xtab[i], ssx, Act.Relu, bias=0.0, scale=-1.0)

        st = [
            opool.tile([P, B, TILES_PER_CORE], fp32, tag=f"st{v}", name=f"st{v}")
            for v in range(4)
        ]
        cls_st = opool.tile([P, B, TILES_PER_CORE], fp32, tag="stc")

        for g in range(GROUPS):
            encc_g = gpool.tile([P, GTILES, 6], fp32, tag="encc")
            nc.sync.dma_start(
                out=encc_g, in_=encc_d[:, g * GTILES * 6 : (g + 1) * GTILES * 6]
            )
            m_acc = gpool.tile([P, GTILES, B, 8], fp32, tag="macc")
            nc.vector.memset(m_acc, -1.0)

            g01_ps = gapool.tile([P, GTILES, 18], fp32, tag="g01")
            g23_ps = gapool.tile([P, GTILES, 18], fp32, tag="g23")

            for gi in range(GTILES):
                slot = g * GTILES + gi
                lv, xt, wyr0 = meta[slot]
                nb = NB[lv]

                # --- wyrep: 3-term bf16 broadcast matmul (sum is ~exact f32) ---
                wyrep_ps = ppool.tile([P, B, N], fp32, tag="wyrep")
                pb = 32 * (slot % 3)
                nc.tensor.matmul(
                    wyrep_ps,
                    onesel[pb : pb + 3 * nb, lv, :],
                    wyrows[pb : pb + 3 * nb, slot // 3, :],
                    start=True,
                    stop=True,
                )
                q = wpool.tile([P, B, N], fp32, tag="q")
                nc.vector.tensor_mul(q, xtab[xt], wyrep_ps)

                # --- max over boxes + argmax + one-hot ---
                nc.vector.tensor_reduce(
                    out=m_acc[:, gi : gi + 1, :, 0:1],
                    in_=q,
                    axis=mybir.AxisListType.X,
                    op=Alu.max,
                )
                i8 = spool.tile([P, B, 8], u32, tag="i8")
                i8f = spool.tile([P, B, 8], fp32, tag="i8f")
                for b in range(B):
                    nc.vector.max_index(i8[:, b, :], m_acc[:, gi, b, :], q[:, b, :])
                nc.vector.tensor_copy(i8f, i8)
                # one-hot in a single broadcast TT
                onehot = wpool.tile([P, B, N], bf16, tag="onehot")
                nc.vector.tensor_tensor(
                    out=onehot,
                    in0=iota.unsqueeze(1).to_broadcast([P, B, N]),
                    in1=i8f[:, :, 0:1].to_broadcast([P, B, N]),
                    op=Alu.is_equal,
                )

                # --- gather via PE: transpose one-hot, matmul with tables ---
                ohT_ps = ppool.tile([P, 2, P], bf16, tag="ohT")
                oh2 = onehot.rearrange("p b n -> p (b n)")
                nc.tensor.transpose(ohT_ps[:, 0, :], oh2[:, 0:P], ident)
                nc.tensor.transpose(ohT_ps[:, 1, :], oh2[:, P : 2 * P], ident)
                ohT = spool.tile([P, 2, P], bf16, tag="ohTs")
                nc.scalar.copy(ohT, ohT_ps)
                nc.tensor.matmul(
                    g01_ps[:, gi, :], ohT[:, 0, :], tables[:, 0:18],
                    start=True, stop=True,
                )
                nc.tensor.matmul(
                    g23_ps[:, gi, :], ohT[:, 1, :], tables[:, 18:36],
                    start=True, stop=True,
                )

            # ---------------- per-group encode ----------------
            # gathered cols per (pair j): j*9 + [cxh,cxl,cyh,cyl,lwh,lwl,lhh,lhl,lab]
            g01 = stpool.tile([P, GTILES, 18], fp32, tag="g01s")
            g23 = stpool.tile([P, GTILES, 18], fp32, tag="g23s")
            nc.scalar.copy(g01, g01_ps)
            nc.scalar.copy(g23, g23_ps)

            gsl = slice(g * GTILES, (g + 1) * GTILES)
            enc = encc_g  # [P, GTILES, 6]: acx, acy, invaw, invah, lwa, lha

            for bp, gsb in ((0, g01), (1, g23)):
                gq = gsb.rearrange("p t (j f) -> p t j f", j=2)  # [P,GT,2,9]

                def outv(tile):
                    return tile[:, 2 * bp : 2 * bp + 2, gsl].transpose([0, 2, 1])

                # center coords: (hi + lo - a) * inv_size
                for v, si in ((0, 2), (1, 3)):
                    hl = stpool.tile([P, GTILES, 2], fp32, tag="enchl")
                    nc.vector.tensor_add(
                        hl, gq[:, :, :, 2 * v], gq[:, :, :, 2 * v + 1]
                    )
                    d = stpool.tile([P, GTILES, 2], fp32, tag="encd")
                    nc.vector.tensor_sub(
                        d, hl, enc[:, :, v : v + 1].to_broadcast([P, GTILES, 2])
                    )
                    nc.vector.tensor_mul(
                        outv(st[v]),
                        d,
                        enc[:, :, si : si + 1].to_broadcast([P, GTILES, 2]),
                    )
                # log-space wh: (lwh + lwl) - log(aw)
                for v, li in ((2, 4), (3, 5)):
                    hl = stpool.tile([P, GTILES, 2], fp32, tag="enchl")
                    nc.vector.tensor_add(
                        hl, gq[:, :, :, 2 * v], gq[:, :, :, 2 * v + 1]
                    )
                    nc.vector.tensor_sub(
                        outv(st[v]),
                        hl,
                        enc[:, :, li : li + 1].to_broadcast([P, GTILES, 2]),
                    )

            # cls: lab*a - (b2 - a);  a = [q >= 1/3], b2 = [q > 2/7]
            mvals = m_acc[:, :, :, 0]  # [P, GTILES, B] stride 8
            amask = stpool.tile([P, GTILES, B], fp32, tag="amask")
            bmask = stpool.tile([P, GTILES, B], fp32, tag="bmask")
            nc.vector.tensor_scalar(
                amask, mvals, float(np.float32(1.0) / np.float32(3.0)), None,
                op0=Alu.is_ge,
            )
            nc.vector.tensor_scalar(
                bmask, mvals, float(np.float32(0.4) / np.float32(1.4)), None,
                op0=Alu.is_gt,
            )
            ba = stpool.tile([P, GTILES, B], fp32, tag="ba")
            nc.vector.tensor_sub(ba, bmask, amask)
            for bp, gsb in ((0, g01), (1, g23)):
                lab = gsb.rearrange("p t (j f) -> p t j f", j=2)[:, :, :, 8]
                la = stpool.tile([P, GTILES, 2], fp32, tag="la")
                nc.vector.tensor_mul(la, lab, amask[:, :, 2 * bp : 2 * bp + 2])
                nc.vector.tensor_sub(
                    cls_st[:, 2 * bp : 2 * bp + 2, gsl].transpose([0, 2, 1]),
                    la,
                    ba[:, :, 2 * bp : 2 * bp + 2],
                )

        # outputs in SBUF-natural order: DRAM[b, p*T + ti] (host unpermutes)
        for v in range(4):
            for b in range(B):
                nc.sync.dma_start(
                    out=loc_d[v][b].rearrange("(p t) -> p t", p=P),
                    in_=st[v][:, b, :],
                )
        for b in range(B):
            nc.sync.dma_start(
                out=cls_d[b].rearrange("(p t) -> p t", p=P),
                in_=cls_st[:, b, :],
            )

    nc.compile()
    return nc


# ----------------------------------------------------------------------------
# Host data preparation
# ----------------------------------------------------------------------------
def _hilo(v):
    import ml_dtypes

    hi = v.astype(ml_dtypes.bfloat16)
    lo = (v - hi.astype(np.float32)).astype(ml_dtypes.bfloat16)
    return hi.astype(np.float32), lo.astype(np.float32)


def _prepare_host(labels, boxes):
    import ml_dtypes

    f32 = np.float32
    bfl = ml_dtypes.bfloat16
    NX = sum(XTILES)

    wh_t = _anchor_wh()  # [4, NT, 2]
    aa_lvl = (wh_t[..., 0] + f32(1.0)) * (wh_t[..., 1] + f32(1.0))  # [4, NT]

    a_, b_ = boxes[..., :2].astype(f32), boxes[..., 2:].astype(f32)
    bxy = (a_ + b_) / f32(2.0)
    bwh = b_ - a_ + f32(1.0)
    b1 = np.concatenate([bxy - bwh / f32(2.0), bxy + bwh / f32(2.0)], -1)  # [B,N,4]
    area_b = (b1[..., 2] - b1[..., 0] + f32(1.0)) * (
        b1[..., 3] - b1[..., 1] + f32(1.0)
    )

    corners = np.empty((P, 2, B, N), f32)
    corners[:, 0] = b1[None, :, :, 0]  # bx1
    corners[:, 1] = b1[None, :, :, 2]  # bx2

    grids = []
    for i in range(4):
        fm = FMS[i]
        grid = S / fm
        grids.append(((np.arange(fm, dtype=f32) + f32(0.5)) * f32(grid)).astype(f32))

    # X-clamp constants per X-tile; pad rows force wx = 0
    xclamp = np.zeros((P, NX, 3), f32)
    xi = 0
    for lv in range(4):
        nb, gxw = NB[lv], GXW[lv]
        for tb in range(TBLK[lv]):
            p = np.arange(P)
            t = tb * nb + p // gxw
            gx = p % gxw
            valid = t < NT
            tcl = np.clip(t, 0, NT - 1)
            w = wh_t[lv, tcl, 0]
            cx = grids[lv][gx]
            ax1 = cx - w / f32(2.0)
            ax2 = cx + w / f32(2.0)
            wa1p = ax2 - ax1 + f32(1.0)
            xclamp[:, xi, 0] = np.where(valid, -ax1, f32(0.0))
            xclamp[:, xi, 1] = np.where(valid, ax2, f32(-1.0e6))
            xclamp[:, xi, 2] = np.where(valid, wa1p, f32(-1.0))
            xi += 1

    # onesel[base + i, lv, p] = 1 where p//gxw == i % nb, for i < 3*nb
    onesel = np.zeros((P, 4, P), f32)
    for base in (0, 32, 64):
        for lv in range(4):
            nb, gxw = NB[lv], GXW[lv]
            for i in range(3 * nb):
                onesel[base + i, lv, (i % nb) * gxw : (i % nb + 1) * gxw] = 1.0

    # gather tables (bf16 hi/lo): [P, 36]
    lw = np.log(bwh[..., 0])
    lh = np.log(bwh[..., 1])
    labf = labels.astype(f32)
    tabs = np.zeros((P, 36), f32)
    for bp in range(2):
        for j in range(2):
            b = 2 * bp + j
            rows = slice(j * N, (j + 1) * N)
            cols = []
            for v in (bxy[b, :, 0], bxy[b, :, 1], lw[b], lh[b]):
                hi, lo = _hilo(v)
                cols += [hi, lo]
            cols.append(labf[b])
            tabs[rows, bp * 18 + j * 9 : bp * 18 + j * 9 + 9] = np.stack(cols, -1)
    tables = tabs.astype(bfl)

    iota = np.broadcast_to(np.arange(N, dtype=f32)[None, :], (P, N)).copy()
    ident = np.eye(P, dtype=f32).astype(bfl)

    anchors = build_anchor_boxes(S)
    acx, acy, aw, ah = anchors[:, 0], anchors[:, 1], anchors[:, 2], anchors[:, 3]
    law, lah = np.log(aw), np.log(ah)
    invaw, invah = f32(1.0) / aw, f32(1.0) / ah

    percore = []
    for c in range(NC_COUNT):
        # wyrows = wy(gy,t) * invS(lv,t) per (b,n); [8, tile, 256] packing
        wyr = np.zeros((8, TILES_PER_CORE, FREE), f32)
        s = 0
        for lv in range(4):
            nb = NB[lv]
            cyv = grids[lv]
            for j in range(CORE_LEVEL_TILES[lv]):
                gtile = c * CORE_LEVEL_TILES[lv] + j
                gy = gtile // TBLK[lv]
                tblk = gtile % TBLK[lv]
                for k in range(nb):
                    t = tblk * nb + k
                    if t < NT:
                        h = wh_t[lv, t, 1]
                        cy = cyv[gy]
                        ay1 = cy - h / f32(2.0)
                        ay2 = cy + h / f32(2.0)
                        ha1p = ay2 - ay1 + f32(1.0)
                        rs = np.maximum(b1[..., 1] - ay1, f32(0.0))
                        rt = np.maximum(ay2 - b1[..., 3], f32(0.0))
                        wy = np.maximum(-((rs - ha1p) + rt), f32(0.0))
                        invs = f32(1.0) / (aa_lvl[lv, t] + area_b)
                        wyr[k, s] = (wy * invs).reshape(-1)
                s += 1
        # 3-term bf16 split packed per tile at partition base 32*(j%4)
        wyp = np.zeros((88, 67, FREE), f32)
        for j in range(TILES_PER_CORE):
            lv = next(
                i for i in range(4)
                if j < sum(CORE_LEVEL_TILES[: i + 1])
            )
            nb = NB[lv]
            base = 32 * (j % 3)
            col = j // 3
            for k in range(nb):
                w = wyr[k, j]
                hi = w.astype(bfl).astype(f32)
                mid = (w - hi).astype(bfl).astype(f32)
                lo = (w - hi - mid).astype(bfl).astype(f32)
                wyp[base + k, col] = hi
                wyp[base + nb + k, col] = mid
                wyp[base + 2 * nb + k, col] = lo
        wyrows = wyp.astype(bfl).reshape(88, 67 * FREE)

        gidx = _core_slot_anchor_index(c)  # [T, P], -1 = pad
        safe = np.where(gidx >= 0, gidx, 0)
        encc = np.stack(
            [acx[safe], acy[safe], invaw[safe], invah[safe], law[safe], lah[safe]],
            -1,
        ).astype(f32)
        encc = np.ascontiguousarray(encc.transpose(1, 0, 2)).reshape(
            P, TILES_PER_CORE * 6
        )

        percore.append(
            dict(
                corners=corners.reshape(P, 2 * FREE),
                xclamp=xclamp.reshape(P, NX * 3),
                encc=encc,
                wyrows=wyrows,
                onesel=onesel.astype(bfl).reshape(P, 4 * P),
                tables=tables,
                iota=iota,
                ident=ident,
            )
        )
    return percore


def _assemble(results, labels_dtype):
    cls_full = np.empty((B, A_TOTAL), np.float32)
    loc_full = np.empty((B, A_TOTAL, 4), np.float32)

    def unperm(a):
        # device emits [B, p*T + ti]; bring to slot-major [B, ti*P + p]
        return np.ascontiguousarray(
            a.reshape(B, P, TILES_PER_CORE).transpose(0, 2, 1).reshape(B, AC)
        )

    for c in range(NC_COUNT):
        r = results[c]
        gidx = _core_slot_anchor_index(c).reshape(-1)
        valid = gidx >= 0
        tgt = gidx[valid]
        cls_full[:, tgt] = unperm(r["clsf"])[:, valid]
        for v in range(4):
            loc_full[:, tgt, v] = unperm(r[f"loc{v}"])[:, valid]
    cls_out = cls_full.astype(labels_dtype)
    return cls_out, loc_full


def _fix_compiler_flags():
    """Skip the neuronxcc DataLocalityOpt pass (crashes on our DMA macros)."""
    from concourse import compiler_utils as cu

    flags = cu.get_compiler_flags()
    out = []
    for f in flags:
        if f.startswith("--tensorizer-options=") and "DataLocalityOpt" not in f:
            f = f.rstrip() + " --skip-pass=DataLocalityOpt "
        out.append(f)
    cu.set_compiler_flags(out)


def _run(labels, boxes, input_size, trace=False):
    from concourse.bass_utils import run_bass_kernel_spmd

    _fix_compiler_flags()

    assert int(input_size) == S, f"kernel hardcoded for input_size={S}"
    labels = np.asarray(labels)
    boxes = np.asarray(boxes, dtype=np.float32)

    if "prog" not in _prog_cache:
        _prog_cache["prog"] = _build_program()
    nc = _prog_cache["prog"]

    in_maps = _prepare_host(labels, boxes)
    res = run_bass_kernel_spmd(
        nc, in_maps, core_ids=list(range(NC_COUNT)), trace=trace
    )
    cls_out, loc_out = _assemble(res.results, labels.dtype)
    return (cls_out, loc_out), res


def kernel(labels, boxes, input_size):
    (cls_out, loc_out), _ = _run(labels, boxes, input_size)
    return cls_out, loc_out


# revision 31
# speedup vs baseline: 1.1978x; 1.0379x over previous
"""Anchor target-assignment (IoU match + encode) on 8 TRN2 NeuronCores.

Self-contained: hardcodes shapes for the nn_Anchors problem
(B=4, N=64, input_size=512, A=195840).

v2 design ("t-major" anchor reorder):
  Anchors are processed in (level, gy, t-block, gx) order so that each
  128-anchor tile has gx on partitions and a constant anchor-type block.
  - x-overlap factors wx(gx,t,b,n) live in 19 precomputed X-table tiles
    [128, 256] (computed on device from box corners, one relu pipeline per
    X-tile instead of per anchor-tile).
  - wy(gy,t,b,n) * invS(lv,t,b,n) is host-computed per tile as 1..8 rows
    [nb, 256]; a tiny PE ones-block matmul broadcasts it to [128, 256].
  - q = wx * (wy*invS) is a per-box monotone transform of IoU
    (q = inter/(area_a+area_b)); argmax_n q == argmax_n IoU and IoU
    thresholds 0.5/0.4 map to q thresholds 1/3 and 2/7.
  - argmax via tensor_reduce(max) + max_index (exact first-occurrence).
  - gather of matched (cx, cy, log w, log h, label) via bf16 one-hot
    PE transpose + matmul; coords/log-sizes are split hi+lo in bf16 to
    keep f32-level accuracy.
  Levels 1-3 pad their t-axis to tile multiples; padded X rows are zero
  (q=0) and the host drops pad slots when stitching.
"""

import math
import os
import sys

import numpy as np

for _p in ("/opt/trn_rl_repo", "/root/.axon_site/_ro/trn_rl_repo"):
    if os.path.isdir(_p) and _p not in sys.path:
        sys.path.insert(0, _p)

# ----------------------------------------------------------------------------
# Problem constants
# ----------------------------------------------------------------------------
B = 4
N = 64
S = 512
ANCHOR_AREAS = [4 * 4, 16 * 16, 64 * 64, 128 * 128]
ASPECT_RATIOS = [1 / 2.0, 1 / 1.0, 2 / 1.0]
SCALE_RATIOS = [1.0, 2.0, 3 / 4.0]
NT = 9  # anchor types per cell
FMS = [128, 64, 32, 16]
LEVEL_ANCHORS = [fm * fm * NT for fm in FMS]  # 147456, 36864, 9216, 2304
A_TOTAL = sum(LEVEL_ANCHORS)  # 195840
P = 128
NC_COUNT = 8
FREE = B * N  # 256

# t-blocking per level: tiles are [NB t-variants x (128/NB) gx cells]
NB = [1, 2, 4, 8]  # t rows per tile (level i)
GXW = [128, 64, 32, 16]  # gx cells per t row (= fm)
TBLK = [9, 5, 3, 2]  # t-blocks per gy row (t padded to NB*TBLK)
LEVEL_TILES = [FMS[i] * TBLK[i] for i in range(4)]  # 1152, 320, 96, 32
CORE_LEVEL_TILES = [t // NC_COUNT for t in LEVEL_TILES]  # 144, 40, 12, 4
TILES_PER_CORE = sum(CORE_LEVEL_TILES)  # 200
AC = TILES_PER_CORE * P  # 25600 slots per core
GROUPS = 8
GTILES = TILES_PER_CORE // GROUPS  # 25
XTILES = TBLK  # X-table tiles per level: 9, 5, 3, 2 -> 19
WYROWS = [CORE_LEVEL_TILES[i] * NB[i] for i in range(4)]  # 144, 80, 48, 32

_prog_cache = {}


# ----------------------------------------------------------------------------
# Host-side anchor construction (mirrors reference.py exactly)
# ----------------------------------------------------------------------------
def _anchor_wh():
    wh = []
    for s in ANCHOR_AREAS:
        for ar in ASPECT_RATIOS:
            h = math.sqrt(s / ar)
            w = ar * h
            for sr in SCALE_RATIOS:
                wh.append([w * sr, h * sr])
    return np.asarray(wh, np.float32).reshape(len(ANCHOR_AREAS), NT, 2)


def build_anchor_boxes(input_size):
    wh = _anchor_wh()
    out = []
    for i in range(len(ANCHOR_AREAS)):
        fm = int(math.ceil(input_size / 2.0 ** (i + 2)))
        grid = input_size / fm
        centers = (np.arange(fm, dtype=np.float32) + 0.5) * grid
        gx, gy = np.meshgrid(centers, centers)
        xy = np.stack([gx, gy], axis=-1)
        xy = np.broadcast_to(xy[:, :, None, :], (fm, fm, NT, 2))
        whl = np.broadcast_to(wh[i][None, None, :, :], (fm, fm, NT, 2))
        out.append(
            np.concatenate([xy, whl], axis=-1).astype(np.float32).reshape(-1, 4)
        )
    return np.concatenate(out, 0)


def _slot_meta():
    """Per-slot (level, xtile_index, wyrow_start) — identical on all cores."""
    meta = []
    for lv in range(4):
        for j in range(CORE_LEVEL_TILES[lv]):
            xt = sum(XTILES[:lv]) + (j % TBLK[lv])
            wyr0 = sum(WYROWS[:lv]) + j * NB[lv]
            meta.append((lv, xt, wyr0))
    return meta


def _core_slot_anchor_index(core):
    """[TILES_PER_CORE, P] global real-anchor index per slot, or -1 for pads."""
    lvl_base = np.cumsum([0] + LEVEL_ANCHORS)
    idx = np.full((TILES_PER_CORE, P), -1, np.int64)
    s = 0
    for lv in range(4):
        nb, gxw, tb = NB[lv], GXW[lv], TBLK[lv]
        for j in range(CORE_LEVEL_TILES[lv]):
            gtile = core * CORE_LEVEL_TILES[lv] + j
            gy = gtile // tb
            tblk = gtile % tb
            p = np.arange(P)
            t = tblk * nb + p // gxw
            gx = p % gxw
            a = lvl_base[lv] + (gy * gxw + gx) * NT + t
            idx[s] = np.where(t < NT, a, -1)
            s += 1
    return idx


# ----------------------------------------------------------------------------
# Bass program (one SPMD program, identical for all 8 cores)
# ----------------------------------------------------------------------------
def _build_program():
    from contextlib import ExitStack

    from concourse import bacc, mybir
    from concourse.tile import TileContext

    fp32 = mybir.dt.float32
    bf16 = mybir.dt.bfloat16
    u32 = mybir.dt.uint32
    Alu = mybir.AluOpType
    Act = mybir.ActivationFunctionType

    nc = bacc.Bacc(None, target_bir_lowering=False)

    NX = sum(XTILES)  # 19

    # inputs
    corners_d = nc.declare_dram_parameter("corners", [P, 2 * FREE], fp32, isOutput=False)
    xclamp_d = nc.declare_dram_parameter("xclamp", [P, NX * 3], fp32, isOutput=False)
    encc_d = nc.declare_dram_parameter("encc", [P, TILES_PER_CORE * 6], fp32, isOutput=False)
    wyr_d = nc.declare_dram_parameter("wyrows", [88, 67 * FREE], bf16, isOutput=False)
    ones_d = nc.declare_dram_parameter("onesel", [P, 4 * P], bf16, isOutput=False)
    tables_d = nc.declare_dram_parameter("tables", [P, 36], bf16, isOutput=False)
    iota_d = nc.declare_dram_parameter("iota", [P, N], fp32, isOutput=False)
    ident_d = nc.declare_dram_parameter("ident", [P, P], bf16, isOutput=False)
    # outputs (v-split loc + float cls), SBUF-natural order (host unpermutes)
    loc_d = [
        nc.declare_dram_parameter(f"loc{v}", [B, AC], fp32, isOutput=True)
        for v in range(4)
    ]
    cls_d = nc.declare_dram_parameter("clsf", [B, AC], fp32, isOutput=True)

    meta = _slot_meta()

    with TileContext(nc) as tc, ExitStack() as ctx:
        cpool = ctx.enter_context(tc.tile_pool(name="consts", bufs=1))
        corners = cpool.tile([P, 2, B, N], fp32)
        xclamp = cpool.tile([P, NX, 3], fp32)
        wyrows = cpool.tile([88, 67, FREE], bf16)
        onesel = cpool.tile([P, 4, P], bf16)
        tables = cpool.tile([P, 36], bf16)
        iota = cpool.tile([P, N], fp32)
        ident = cpool.tile([P, P], bf16)
        nc.sync.dma_start(out=corners, in_=corners_d[:, :])
        nc.sync.dma_start(out=xclamp, in_=xclamp_d[:, :])
        nc.sync.dma_start(out=wyrows, in_=wyr_d[:, :])
        nc.sync.dma_start(out=onesel, in_=ones_d[:, :])
        nc.sync.dma_start(out=tables, in_=tables_d[:, :])
        nc.sync.dma_start(out=iota, in_=iota_d[:, :])
        nc.sync.dma_start(out=ident, in_=ident_d[:, :])

        xpool = ctx.enter_context(tc.tile_pool(name="xtab", bufs=1))
        xprep = ctx.enter_context(tc.tile_pool(name="xprep", bufs=3))
        gpool = ctx.enter_context(tc.tile_pool(name="group", bufs=2))
        wpool = ctx.enter_context(tc.tile_pool(name="work", bufs=6))
        spool = ctx.enter_context(tc.tile_pool(name="small", bufs=6))
        ppool = ctx.enter_context(tc.tile_pool(name="psum", bufs=3, space="PSUM"))
        gapool = ctx.enter_context(tc.tile_pool(name="gacc", bufs=1, space="PSUM"))
        stpool = ctx.enter_context(tc.tile_pool(name="stage", bufs=2))
        opool = ctx.enter_context(tc.tile_pool(name="outstage", bufs=1))

        # ---------------- X-table prep: 19 tiles ----------------
        xtab = [
            xpool.tile([P, B, N], fp32, tag=f"x{i}", name=f"x{i}") for i in range(NX)
        ]
        for i in range(NX):
            nax1 = xclamp[:, i : i + 1, 0:1]
            ax2 = xclamp[:, i : i + 1, 1:2]
            wa1p = xclamp[:, i : i + 1, 2:3]
            rsx = xprep.tile([P, B, N], fp32, tag="rsx")
            rtx = xprep.tile([P, B, N], fp32, tag="rtx")
            nc.scalar.activation(rsx, corners[:, 0], Act.Relu, bias=nax1, scale=1.0)
            nc.scalar.activation(rtx, corners[:, 1], Act.Relu, bias=ax2, scale=-1.0)
            ssx = xprep.tile([P, B, N], fp32, tag="ssx")
            nc.vector.scalar_tensor_tensor(
                ssx, rsx, wa1p, rtx, op0=Alu.subtract, op1=Alu.add
            )
            nc.scalar.activation(xtab[i], ssx, Act.Relu, bias=0.0, scale=-1.0)

        st = [
            opool.tile([P, B, TILES_PER_CORE], fp32, tag=f"st{v}", name=f"st{v}")
            for v in range(4)
        ]
        cls_st = opool.tile([P, B, TILES_PER_CORE], fp32, tag="stc")

        for g in range(GROUPS):
            encc_g = gpool.tile([P, GTILES, 6], fp32, tag="encc")
            nc.sync.dma_start(
                out=encc_g, in_=encc_d[:, g * GTILES * 6 : (g + 1) * GTILES * 6]
            )
            m_acc = gpool.tile([P, GTILES, B, 8], fp32, tag="macc")
            nc.vector.memset(m_acc, -1.0)

            g01_ps = gapool.tile([P, GTILES, 18], fp32, tag="g01")
            g23_ps = gapool.tile([P, GTILES, 18], fp32, tag="g23")

            for gi in range(GTILES):
                slot = g * GTILES + gi
                lv, xt, wyr0 = meta[slot]
                nb = NB[lv]

                # --- wyrep: 3-term bf16 broadcast matmul (sum is ~exact f32) ---
                wyrep_ps = ppool.tile([P, B, N], fp32, tag="wyrep")
                pb = 32 * (slot % 3)
                nc.tensor.matmul(
                    wyrep_ps,
                    onesel[pb : pb + 3 * nb, lv, :],
                    wyrows[pb : pb + 3 * nb, slot // 3, :],
                    start=True,
                    stop=True,
                )
                q = wpool.tile([P, B, N], fp32, tag="q")
                nc.vector.tensor_mul(q, xtab[xt], wyrep_ps)

                # --- max over boxes + argmax + one-hot ---
                nc.vector.tensor_reduce(
                    out=m_acc[:, gi : gi + 1, :, 0:1],
                    in_=q,
                    axis=mybir.AxisListType.X,
                    op=Alu.max,
                )
                i8 = spool.tile([P, B, 8], u32, tag="i8")
                i8f = spool.tile([P, B, 8], fp32, tag="i8f")
                for b in range(B):
                    nc.vector.max_index(i8[:, b, :], m_acc[:, gi, b, :], q[:, b, :])
                nc.scalar.copy(i8f, i8)
                # one-hot in a single broadcast TT
                onehot = wpool.tile([P, B, N], bf16, tag="onehot")
                nc.vector.tensor_tensor(
                    out=onehot,
                    in0=iota.unsqueeze(1).to_broadcast([P, B, N]),
                    in1=i8f[:, :, 0:1].to_broadcast([P, B, N]),
                    op=Alu.is_equal,
                )

                # --- gather via PE: transpose one-hot, matmul with tables ---
                ohT_ps = ppool.tile([P, 2, P], bf16, tag="ohT")
                oh2 = onehot.rearrange("p b n -> p (b n)")
                nc.tensor.transpose(ohT_ps[:, 0, :], oh2[:, 0:P], ident)
                nc.tensor.transpose(ohT_ps[:, 1, :], oh2[:, P : 2 * P], ident)
                ohT = spool.tile([P, 2, P], bf16, tag="ohTs")
                nc.scalar.copy(ohT, ohT_ps)
                nc.tensor.matmul(
                    g01_ps[:, gi, :], ohT[:, 0, :], tables[:, 0:18],
                    start=True, stop=True,
                )
                nc.tensor.matmul(
                    g23_ps[:, gi, :], ohT[:, 1, :], tables[:, 18:36],
                    start=True, stop=True,
                )

            # ---------------- per-group encode ----------------
            # gathered cols per (pair j): j*9 + [cxh,cxl,cyh,cyl,lwh,lwl,lhh,lhl,lab]
            g01 = stpool.tile([P, GTILES, 18], fp32, tag="g01s")
            g23 = stpool.tile([P, GTILES, 18], fp32, tag="g23s")
            nc.scalar.copy(g01, g01_ps)
            nc.scalar.copy(g23, g23_ps)

            gsl = slice(g * GTILES, (g + 1) * GTILES)
            enc = encc_g  # [P, GTILES, 6]: acx, acy, invaw, invah, lwa, lha

            for bp, gsb in ((0, g01), (1, g23)):
                gq = gsb.rearrange("p t (j f) -> p t j f", j=2)  # [P,GT,2,9]

                def outv(tile):
                    return tile[:, 2 * bp : 2 * bp + 2, gsl].transpose([0, 2, 1])

                # center coords: (hi + lo - a) * inv_size
                for v, si in ((0, 2), (1, 3)):
                    hl = stpool.tile([P, GTILES, 2], fp32, tag="enchl")
                    nc.vector.tensor_add(
                        hl, gq[:, :, :, 2 * v], gq[:, :, :, 2 * v + 1]
                    )
                    d = stpool.tile([P, GTILES, 2], fp32, tag="encd")
                    nc.vector.tensor_sub(
                        d, hl, enc[:, :, v : v + 1].to_broadcast([P, GTILES, 2])
                    )
                    nc.vector.tensor_mul(
                        outv(st[v]),
                        d,
                        enc[:, :, si : si + 1].to_broadcast([P, GTILES, 2]),
                    )
                # log-space wh: (lwh + lwl) - log(aw)
                for v, li in ((2, 4), (3, 5)):
                    hl = stpool.tile([P, GTILES, 2], fp32, tag="enchl")
                    nc.vector.tensor_add(
                        hl, gq[:, :, :, 2 * v], gq[:, :, :, 2 * v + 1]
                    )
                    nc.vector.tensor_sub(
                        outv(st[v]),
                        hl,
                        enc[:, :, li : li + 1].to_broadcast([P, GTILES, 2]),
                    )

            # cls: lab*a - (b2 - a);  a = [q >= 1/3], b2 = [q > 2/7]
            mvals = m_acc[:, :, :, 0]  # [P, GTILES, B] stride 8
            amask = stpool.tile([P, GTILES, B], fp32, tag="amask")
            bmask = stpool.tile([P, GTILES, B], fp32, tag="bmask")
            nc.vector.tensor_scalar(
                amask, mvals, float(np.float32(1.0) / np.float32(3.0)), None,
                op0=Alu.is_ge,
            )
            nc.vector.tensor_scalar(
                bmask, mvals, float(np.float32(0.4) / np.float32(1.4)), None,
                op0=Alu.is_gt,
            )
            ba = stpool.tile([P, GTILES, B], fp32, tag="ba")
            nc.vector.tensor_sub(ba, bmask, amask)
            for bp, gsb in ((0, g01), (1, g23)):
                lab = gsb.rearrange("p t (j f) -> p t j f", j=2)[:, :, :, 8]
                la = stpool.tile([P, GTILES, 2], fp32, tag="la")
                nc.vector.tensor_mul(la, lab, amask[:, :, 2 * bp : 2 * bp + 2])
                nc.vector.tensor_sub(
                    cls_st[:, 2 * bp : 2 * bp + 2, gsl].transpose([0, 2, 1]),
                    la,
                    ba[:, :, 2 * bp : 2 * bp + 2],
                )

        # outputs in SBUF-natural order: DRAM[b, p*T + ti] (host unpermutes)
        for v in range(4):
            for b in range(B):
                nc.sync.dma_start(
                    out=loc_d[v][b].rearrange("(p t) -> p t", p=P),
                    in_=st[v][:, b, :],
                )
        for b in range(B):
            nc.sync.dma_start(
                out=cls_d[b].rearrange("(p t) -> p t", p=P),
                in_=cls_st[:, b, :],
            )

    nc.compile()
    return nc


# ----------------------------------------------------------------------------
# Host data preparation
# ----------------------------------------------------------------------------
def _hilo(v):
    import ml_dtypes

    hi = v.astype(ml_dtypes.bfloat16)
    lo = (v - hi.astype(np.float32)).astype(ml_dtypes.bfloat16)
    return hi.astype(np.float32), lo.astype(np.float32)


def _prepare_host(labels, boxes):
    import ml_dtypes

    f32 = np.float32
    bfl = ml_dtypes.bfloat16
    NX = sum(XTILES)

    wh_t = _anchor_wh()  # [4, NT, 2]
    aa_lvl = (wh_t[..., 0] + f32(1.0)) * (wh_t[..., 1] + f32(1.0))  # [4, NT]

    a_, b_ = boxes[..., :2].astype(f32), boxes[..., 2:].astype(f32)
    bxy = (a_ + b_) / f32(2.0)
    bwh = b_ - a_ + f32(1.0)
    b1 = np.concatenate([bxy - bwh / f32(2.0), bxy + bwh / f32(2.0)], -1)  # [B,N,4]
    area_b = (b1[..., 2] - b1[..., 0] + f32(1.0)) * (
        b1[..., 3] - b1[..., 1] + f32(1.0)
    )

    corners = np.empty((P, 2, B, N), f32)
    corners[:, 0] = b1[None, :, :, 0]  # bx1
    corners[:, 1] = b1[None, :, :, 2]  # bx2

    grids = []
    for i in range(4):
        fm = FMS[i]
        grid = S / fm
        grids.append(((np.arange(fm, dtype=f32) + f32(0.5)) * f32(grid)).astype(f32))

    # X-clamp constants per X-tile; pad rows force wx = 0
    xclamp = np.zeros((P, NX, 3), f32)
    xi = 0
    for lv in range(4):
        nb, gxw = NB[lv], GXW[lv]
        for tb in range(TBLK[lv]):
            p = np.arange(P)
            t = tb * nb + p // gxw
            gx = p % gxw
            valid = t < NT
            tcl = np.clip(t, 0, NT - 1)
            w = wh_t[lv, tcl, 0]
            cx = grids[lv][gx]
            ax1 = cx - w / f32(2.0)
            ax2 = cx + w / f32(2.0)
            wa1p = ax2 - ax1 + f32(1.0)
            xclamp[:, xi, 0] = np.where(valid, -ax1, f32(0.0))
            xclamp[:, xi, 1] = np.where(valid, ax2, f32(-1.0e6))
            xclamp[:, xi, 2] = np.where(valid, wa1p, f32(-1.0))
            xi += 1

    # onesel[base + i, lv, p] = 1 where p//gxw == i % nb, for i < 3*nb
    onesel = np.zeros((P, 4, P), f32)
    for base in (0, 32, 64):
        for lv in range(4):
            nb, gxw = NB[lv], GXW[lv]
            for i in range(3 * nb):
                onesel[base + i, lv, (i % nb) * gxw : (i % nb + 1) * gxw] = 1.0

    # gather tables (bf16 hi/lo): [P, 36]
    lw = np.log(bwh[..., 0])
    lh = np.log(bwh[..., 1])
    labf = labels.astype(f32)
    tabs = np.zeros((P, 36), f32)
    for bp in range(2):
        for j in range(2):
            b = 2 * bp + j
            rows = slice(j * N, (j + 1) * N)
            cols = []
            for v in (bxy[b, :, 0], bxy[b, :, 1], lw[b], lh[b]):
                hi, lo = _hilo(v)
                cols += [hi, lo]
            cols.append(labf[b])
            tabs[rows, bp * 18 + j * 9 : bp * 18 + j * 9 + 9] = np.stack(cols, -1)
    tables = tabs.astype(bfl)

    iota = np.broadcast_to(np.arange(N, dtype=f32)[None, :], (P, N)).copy()
    ident = np.eye(P, dtype=f32).astype(bfl)

    anchors = build_anchor_boxes(S)
    acx, acy, aw, ah = anchors[:, 0], anchors[:, 1], anchors[:, 2], anchors[:, 3]
    law, lah = np.log(aw), np.log(ah)
    invaw, invah = f32(1.0) / aw, f32(1.0) / ah

    percore = []
    for c in range(NC_COUNT):
        # wyrows = wy(gy,t) * invS(lv,t) per (b,n); [8, tile, 256] packing
        wyr = np.zeros((8, TILES_PER_CORE, FREE), f32)
        s = 0
        for lv in range(4):
            nb = NB[lv]
            cyv = grids[lv]
            for j in range(CORE_LEVEL_TILES[lv]):
                gtile = c * CORE_LEVEL_TILES[lv] + j
                gy = gtile // TBLK[lv]
                tblk = gtile % TBLK[lv]
                for k in range(nb):
                    t = tblk * nb + k
                    if t < NT:
                        h = wh_t[lv, t, 1]
                        cy = cyv[gy]
                        ay1 = cy - h / f32(2.0)
                        ay2 = cy + h / f32(2.0)
                        ha1p = ay2 - ay1 + f32(1.0)
                        rs = np.maximum(b1[..., 1] - ay1, f32(0.0))
                        rt = np.maximum(ay2 - b1[..., 3], f32(0.0))
                        wy = np.maximum(-((rs - ha1p) + rt), f32(0.0))
                        invs = f32(1.0) / (aa_lvl[lv, t] + area_b)
                        wyr[k, s] = (wy * invs).reshape(-1)
                s += 1
        # 3-term bf16 split packed per tile at partition base 32*(j%4)
        wyp = np.zeros((88, 67, FREE), f32)
        for j in range(TILES_PER_CORE):
            lv = next(
                i for i in range(4)
                if j < sum(CORE_LEVEL_TILES[: i + 1])
            )
            nb = NB[lv]
            base = 32 * (j % 3)
            col = j // 3
            for k in range(nb):
                w = wyr[k, j]
                hi = w.astype(bfl).astype(f32)
                mid = (w - hi).astype(bfl).astype(f32)
                lo = (w - hi - mid).astype(bfl).astype(f32)
                wyp[base + k, col] = hi
                wyp[base + nb + k, col] = mid
                wyp[base + 2 * nb + k, col] = lo
        wyrows = wyp.astype(bfl).reshape(88, 67 * FREE)

        gidx = _core_slot_anchor_index(c)  # [T, P], -1 = pad
        safe = np.where(gidx >= 0, gidx, 0)
        encc = np.stack(
            [acx[safe], acy[safe], invaw[safe], invah[safe], law[safe], lah[safe]],
            -1,
        ).astype(f32)
        encc = np.ascontiguousarray(encc.transpose(1, 0, 2)).reshape(
            P, TILES_PER_CORE * 6
        )

        percore.append(
            dict(
                corners=corners.reshape(P, 2 * FREE),
                xclamp=xclamp.reshape(P, NX * 3),
                encc=encc,
                wyrows=wyrows,
                onesel=onesel.astype(bfl).reshape(P, 4 * P),
                tables=tables,
                iota=iota,
                ident=ident,
            )
        )
    return percore


def _assemble(results, labels_dtype):
    cls_full = np.empty((B, A_TOTAL), np.float32)
    loc_full = np.empty((B, A_TOTAL, 4), np.float32)

    def unperm(a):
        # device emits [B, p*T + ti]; bring to slot-major [B, ti*P + p]
        return np.ascontiguousarray(
            a.reshape(B, P, TILES_PER_CORE).transpose(0, 2, 1).reshape(B, AC)
        )

    for c in range(NC_COUNT):
        r = results[c]
        gidx = _core_slot_anchor_index(c).reshape(-1)
        valid = gidx >= 0
        tgt = gidx[valid]
        cls_full[:, tgt] = unperm(r["clsf"])[:, valid]
        for v in range(4):
            loc_full[:, tgt, v] = unperm(r[f"loc{v}"])[:, valid]
    cls_out = cls_full.astype(labels_dtype)
    return cls_out, loc_full


def _fix_compiler_flags():
    """Skip the neuronxcc DataLocalityOpt pass (crashes on our DMA macros)."""
    from concourse import compiler_utils as cu

    flags = cu.get_compiler_flags()
    out = []
    for f in flags:
        if f.startswith("--tensorizer-options=") and "DataLocalityOpt" not in f:
            f = f.rstrip() + " --skip-pass=DataLocalityOpt "
        out.append(f)
    cu.set_compiler_flags(out)


def _run(labels, boxes, input_size, trace=False):
    from concourse.bass_utils import run_bass_kernel_spmd

    _fix_compiler_flags()

    assert int(input_size) == S, f"kernel hardcoded for input_size={S}"
    labels = np.asarray(labels)
    boxes = np.asarray(boxes, dtype=np.float32)

    if "prog" not in _prog_cache:
        _prog_cache["prog"] = _build_program()
    nc = _prog_cache["prog"]

    in_maps = _prepare_host(labels, boxes)
    res = run_bass_kernel_spmd(
        nc, in_maps, core_ids=list(range(NC_COUNT)), trace=trace
    )
    cls_out, loc_out = _assemble(res.results, labels.dtype)
    return (cls_out, loc_out), res


def kernel(labels, boxes, input_size):
    (cls_out, loc_out), _ = _run(labels, boxes, input_size)
    return cls_out, loc_out


# revision 32
# speedup vs baseline: 1.2094x; 1.0096x over previous
"""Anchor target-assignment (IoU match + encode) on 8 TRN2 NeuronCores.

Self-contained: hardcodes shapes for the nn_Anchors problem
(B=4, N=64, input_size=512, A=195840).

v2 design ("t-major" anchor reorder):
  Anchors are processed in (level, gy, t-block, gx) order so that each
  128-anchor tile has gx on partitions and a constant anchor-type block.
  - x-overlap factors wx(gx,t,b,n) live in 19 precomputed X-table tiles
    [128, 256] (computed on device from box corners, one relu pipeline per
    X-tile instead of per anchor-tile).
  - wy(gy,t,b,n) * invS(lv,t,b,n) is host-computed per tile as 1..8 rows
    [nb, 256]; a tiny PE ones-block matmul broadcasts it to [128, 256].
  - q = wx * (wy*invS) is a per-box monotone transform of IoU
    (q = inter/(area_a+area_b)); argmax_n q == argmax_n IoU and IoU
    thresholds 0.5/0.4 map to q thresholds 1/3 and 2/7.
  - argmax via tensor_reduce(max) + max_index (exact first-occurrence).
  - gather of matched (cx, cy, log w, log h, label) via bf16 one-hot
    PE transpose + matmul; coords/log-sizes are split hi+lo in bf16 to
    keep f32-level accuracy.
  Levels 1-3 pad their t-axis to tile multiples; padded X rows are zero
  (q=0) and the host drops pad slots when stitching.
"""

import math
import os
import sys

import numpy as np

for _p in ("/opt/trn_rl_repo", "/root/.axon_site/_ro/trn_rl_repo"):
    if os.path.isdir(_p) and _p not in sys.path:
        sys.path.insert(0, _p)

# ----------------------------------------------------------------------------
# Problem constants
# ----------------------------------------------------------------------------
B = 4
N = 64
S = 512
ANCHOR_AREAS = [4 * 4, 16 * 16, 64 * 64, 128 * 128]
ASPECT_RATIOS = [1 / 2.0, 1 / 1.0, 2 / 1.0]
SCALE_RATIOS = [1.0, 2.0, 3 / 4.0]
NT = 9  # anchor types per cell
FMS = [128, 64, 32, 16]
LEVEL_ANCHORS = [fm * fm * NT for fm in FMS]  # 147456, 36864, 9216, 2304
A_TOTAL = sum(LEVEL_ANCHORS)  # 195840
P = 128
NC_COUNT = 8
FREE = B * N  # 256

# t-blocking per level: tiles are [NB t-variants x (128/NB) gx cells]
NB = [1, 2, 4, 8]  # t rows per tile (level i)
GXW = [128, 64, 32, 16]  # gx cells per t row (= fm)
TBLK = [9, 5, 3, 2]  # t-blocks per gy row (t padded to NB*TBLK)
LEVEL_TILES = [FMS[i] * TBLK[i] for i in range(4)]  # 1152, 320, 96, 32
CORE_LEVEL_TILES = [t // NC_COUNT for t in LEVEL_TILES]  # 144, 40, 12, 4
TILES_PER_CORE = sum(CORE_LEVEL_TILES)  # 200
AC = TILES_PER_CORE * P  # 25600 slots per core
GROUPS = 8
GTILES = TILES_PER_CORE // GROUPS  # 25
XTILES = TBLK  # X-table tiles per level: 9, 5, 3, 2 -> 19
WYROWS = [CORE_LEVEL_TILES[i] * NB[i] for i in range(4)]  # 144, 80, 48, 32

_prog_cache = {}


# ----------------------------------------------------------------------------
# Host-side anchor construction (mirrors reference.py exactly)
# ----------------------------------------------------------------------------
def _anchor_wh():
    wh = []
    for s in ANCHOR_AREAS:
        for ar in ASPECT_RATIOS:
            h = math.sqrt(s / ar)
            w = ar * h
            for sr in SCALE_RATIOS:
                wh.append([w * sr, h * sr])
    return np.asarray(wh, np.float32).reshape(len(ANCHOR_AREAS), NT, 2)


def build_anchor_boxes(input_size):
    wh = _anchor_wh()
    out = []
    for i in range(len(ANCHOR_AREAS)):
        fm = int(math.ceil(input_size / 2.0 ** (i + 2)))
        grid = input_size / fm
        centers = (np.arange(fm, dtype=np.float32) + 0.5) * grid
        gx, gy = np.meshgrid(centers, centers)
        xy = np.stack([gx, gy], axis=-1)
        xy = np.broadcast_to(xy[:, :, None, :], (fm, fm, NT, 2))
        whl = np.broadcast_to(wh[i][None, None, :, :], (fm, fm, NT, 2))
        out.append(
            np.concatenate([xy, whl], axis=-1).astype(np.float32).reshape(-1, 4)
        )
    return np.concatenate(out, 0)


def _slot_meta():
    """Per-slot (level, xtile_index, wyrow_start) — identical on all cores."""
    meta = []
    for lv in range(4):
        for j in range(CORE_LEVEL_TILES[lv]):
            xt = sum(XTILES[:lv]) + (j % TBLK[lv])
            wyr0 = sum(WYROWS[:lv]) + j * NB[lv]
            meta.append((lv, xt, wyr0))
    return meta


def _core_slot_anchor_index(core):
    """[TILES_PER_CORE, P] global real-anchor index per slot, or -1 for pads."""
    lvl_base = np.cumsum([0] + LEVEL_ANCHORS)
    idx = np.full((TILES_PER_CORE, P), -1, np.int64)
    s = 0
    for lv in range(4):
        nb, gxw, tb = NB[lv], GXW[lv], TBLK[lv]
        for j in range(CORE_LEVEL_TILES[lv]):
            gtile = core * CORE_LEVEL_TILES[lv] + j
            gy = gtile // tb
            tblk = gtile % tb
            p = np.arange(P)
            t = tblk * nb + p // gxw
            gx = p % gxw
            a = lvl_base[lv] + (gy * gxw + gx) * NT + t
            idx[s] = np.where(t < NT, a, -1)
            s += 1
    return idx


# ----------------------------------------------------------------------------
# Bass program (one SPMD program, identical for all 8 cores)
# ----------------------------------------------------------------------------
def _build_program():
    from contextlib import ExitStack

    from concourse import bacc, mybir
    from concourse.tile import TileContext

    fp32 = mybir.dt.float32
    bf16 = mybir.dt.bfloat16
    u32 = mybir.dt.uint32
    Alu = mybir.AluOpType
    Act = mybir.ActivationFunctionType

    nc = bacc.Bacc(None, target_bir_lowering=False)

    NX = sum(XTILES)  # 19

    # inputs
    corners_d = nc.declare_dram_parameter("corners", [P, 2 * FREE], fp32, isOutput=False)
    xclamp_d = nc.declare_dram_parameter("xclamp", [P, NX * 3], fp32, isOutput=False)
    encc_d = nc.declare_dram_parameter("encc", [P, TILES_PER_CORE * 6], fp32, isOutput=False)
    wyr_d = nc.declare_dram_parameter("wyrows", [88, 67 * FREE], bf16, isOutput=False)
    ones_d = nc.declare_dram_parameter("onesel", [P, 4 * P], bf16, isOutput=False)
    tables_d = nc.declare_dram_parameter("tables", [P, 36], bf16, isOutput=False)
    iota_d = nc.declare_dram_parameter("iota", [P, N], fp32, isOutput=False)
    ident_d = nc.declare_dram_parameter("ident", [P, P], bf16, isOutput=False)
    # outputs (v-split loc + float cls), SBUF-natural order (host unpermutes)
    loc_d = [
        nc.declare_dram_parameter(f"loc{v}", [B, AC], fp32, isOutput=True)
        for v in range(4)
    ]
    cls_d = nc.declare_dram_parameter("clsf", [B, AC], fp32, isOutput=True)

    meta = _slot_meta()

    with TileContext(nc) as tc, ExitStack() as ctx:
        cpool = ctx.enter_context(tc.tile_pool(name="consts", bufs=1))
        corners = cpool.tile([P, 2, B, N], fp32)
        xclamp = cpool.tile([P, NX, 3], fp32)
        wyrows = cpool.tile([88, 67, FREE], bf16)
        onesel = cpool.tile([P, 4, P], bf16)
        tables = cpool.tile([P, 36], bf16)
        iota = cpool.tile([P, N], fp32)
        ident = cpool.tile([P, P], bf16)
        nc.sync.dma_start(out=corners, in_=corners_d[:, :])
        nc.sync.dma_start(out=xclamp, in_=xclamp_d[:, :])
        nc.sync.dma_start(out=wyrows, in_=wyr_d[:, :])
        nc.sync.dma_start(out=onesel, in_=ones_d[:, :])
        nc.sync.dma_start(out=tables, in_=tables_d[:, :])
        nc.sync.dma_start(out=iota, in_=iota_d[:, :])
        nc.sync.dma_start(out=ident, in_=ident_d[:, :])

        xpool = ctx.enter_context(tc.tile_pool(name="xtab", bufs=1))
        xprep = ctx.enter_context(tc.tile_pool(name="xprep", bufs=6))
        gpool = ctx.enter_context(tc.tile_pool(name="group", bufs=3))
        wpool = ctx.enter_context(tc.tile_pool(name="work", bufs=6))
        spool = ctx.enter_context(tc.tile_pool(name="small", bufs=6))
        ppool = ctx.enter_context(tc.tile_pool(name="psum", bufs=3, space="PSUM"))
        gapool = ctx.enter_context(tc.tile_pool(name="gacc", bufs=1, space="PSUM"))
        stpool = ctx.enter_context(tc.tile_pool(name="stage", bufs=2))
        opool = ctx.enter_context(tc.tile_pool(name="outstage", bufs=1))

        # ---------------- X-table prep: 19 tiles ----------------
        xtab = [
            xpool.tile([P, B, N], fp32, tag=f"x{i}", name=f"x{i}") for i in range(NX)
        ]
        for i in range(NX):
            nax1 = xclamp[:, i : i + 1, 0:1]
            ax2 = xclamp[:, i : i + 1, 1:2]
            wa1p = xclamp[:, i : i + 1, 2:3]
            rsx = xprep.tile([P, B, N], fp32, tag="rsx")
            rtx = xprep.tile([P, B, N], fp32, tag="rtx")
            nc.scalar.activation(rsx, corners[:, 0], Act.Relu, bias=nax1, scale=1.0)
            nc.scalar.activation(rtx, corners[:, 1], Act.Relu, bias=ax2, scale=-1.0)
            ssx = xprep.tile([P, B, N], fp32, tag="ssx")
            nc.vector.scalar_tensor_tensor(
                ssx, rsx, wa1p, rtx, op0=Alu.subtract, op1=Alu.add
            )
            nc.scalar.activation(xtab[i], ssx, Act.Relu, bias=0.0, scale=-1.0)

        st = [
            opool.tile([P, B, TILES_PER_CORE], fp32, tag=f"st{v}", name=f"st{v}")
            for v in range(4)
        ]
        cls_st = opool.tile([P, B, TILES_PER_CORE], fp32, tag="stc")

        for g in range(GROUPS):
            encc_g = gpool.tile([P, GTILES, 6], fp32, tag="encc")
            nc.sync.dma_start(
                out=encc_g, in_=encc_d[:, g * GTILES * 6 : (g + 1) * GTILES * 6]
            )
            m_acc = gpool.tile([P, GTILES, B, 8], fp32, tag="macc")
            nc.gpsimd.memset(m_acc, -1.0)

            g01_ps = gapool.tile([P, GTILES, 18], fp32, tag="g01")
            g23_ps = gapool.tile([P, GTILES, 18], fp32, tag="g23")

            for gi in range(GTILES):
                slot = g * GTILES + gi
                lv, xt, wyr0 = meta[slot]
                nb = NB[lv]

                # --- wyrep: 3-term bf16 broadcast matmul (sum is ~exact f32) ---
                wyrep_ps = ppool.tile([P, B, N], fp32, tag="wyrep")
                pb = 32 * (slot % 3)
                nc.tensor.matmul(
                    wyrep_ps,
                    onesel[pb : pb + 3 * nb, lv, :],
                    wyrows[pb : pb + 3 * nb, slot // 3, :],
                    start=True,
                    stop=True,
                )
                q = wpool.tile([P, B, N], fp32, tag="q")
                nc.vector.tensor_mul(q, xtab[xt], wyrep_ps)

                # --- max over boxes + argmax + one-hot ---
                nc.vector.tensor_reduce(
                    out=m_acc[:, gi : gi + 1, :, 0:1],
                    in_=q,
                    axis=mybir.AxisListType.X,
                    op=Alu.max,
                )
                i8 = spool.tile([P, B, 8], u32, tag="i8")
                i8f = spool.tile([P, B, 8], fp32, tag="i8f")
                for b in range(B):
                    nc.vector.max_index(i8[:, b, :], m_acc[:, gi, b, :], q[:, b, :])
                nc.scalar.copy(i8f, i8)
                # one-hot in a single broadcast TT
                onehot = wpool.tile([P, B, N], bf16, tag="onehot")
                nc.vector.tensor_tensor(
                    out=onehot,
                    in0=iota.unsqueeze(1).to_broadcast([P, B, N]),
                    in1=i8f[:, :, 0:1].to_broadcast([P, B, N]),
                    op=Alu.is_equal,
                )

                # --- gather via PE: transpose one-hot, matmul with tables ---
                ohT_ps = ppool.tile([P, 2, P], bf16, tag="ohT")
                oh2 = onehot.rearrange("p b n -> p (b n)")
                nc.tensor.transpose(ohT_ps[:, 0, :], oh2[:, 0:P], ident)
                nc.tensor.transpose(ohT_ps[:, 1, :], oh2[:, P : 2 * P], ident)
                ohT = spool.tile([P, 2, P], bf16, tag="ohTs")
                nc.scalar.copy(ohT, ohT_ps)
                nc.tensor.matmul(
                    g01_ps[:, gi, :], ohT[:, 0, :], tables[:, 0:18],
                    start=True, stop=True,
                )
                nc.tensor.matmul(
                    g23_ps[:, gi, :], ohT[:, 1, :], tables[:, 18:36],
                    start=True, stop=True,
                )

            # ---------------- per-group encode ----------------
            # gathered cols per (pair j): j*9 + [cxh,cxl,cyh,cyl,lwh,lwl,lhh,lhl,lab]
            g01 = stpool.tile([P, GTILES, 18], fp32, tag="g01s")
            g23 = stpool.tile([P, GTILES, 18], fp32, tag="g23s")
            nc.scalar.copy(g01, g01_ps)
            nc.scalar.copy(g23, g23_ps)

            gsl = slice(g * GTILES, (g + 1) * GTILES)
            enc = encc_g  # [P, GTILES, 6]: acx, acy, invaw, invah, lwa, lha

            for bp, gsb in ((0, g01), (1, g23)):
                gq = gsb.rearrange("p t (j f) -> p t j f", j=2)  # [P,GT,2,9]

                def outv(tile):
                    return tile[:, 2 * bp : 2 * bp + 2, gsl].transpose([0, 2, 1])

                # center coords: (hi + lo - a) * inv_size
                for v, si in ((0, 2), (1, 3)):
                    hl = stpool.tile([P, GTILES, 2], fp32, tag="enchl")
                    nc.vector.tensor_add(
                        hl, gq[:, :, :, 2 * v], gq[:, :, :, 2 * v + 1]
                    )
                    d = stpool.tile([P, GTILES, 2], fp32, tag="encd")
                    nc.vector.tensor_sub(
                        d, hl, enc[:, :, v : v + 1].to_broadcast([P, GTILES, 2])
                    )
                    nc.vector.tensor_mul(
                        outv(st[v]),
                        d,
                        enc[:, :, si : si + 1].to_broadcast([P, GTILES, 2]),
                    )
                # log-space wh: (lwh + lwl) - log(aw)
                for v, li in ((2, 4), (3, 5)):
                    hl = stpool.tile([P, GTILES, 2], fp32, tag="enchl")
                    nc.vector.tensor_add(
                        hl, gq[:, :, :, 2 * v], gq[:, :, :, 2 * v + 1]
                    )
                    nc.vector.tensor_sub(
                        outv(st[v]),
                        hl,
                        enc[:, :, li : li + 1].to_broadcast([P, GTILES, 2]),
                    )

            # cls: lab*a - (b2 - a);  a = [q >= 1/3], b2 = [q > 2/7]
            mvals = m_acc[:, :, :, 0]  # [P, GTILES, B] stride 8
            amask = stpool.tile([P, GTILES, B], fp32, tag="amask")
            bmask = stpool.tile([P, GTILES, B], fp32, tag="bmask")
            nc.vector.tensor_scalar(
                amask, mvals, float(np.float32(1.0) / np.float32(3.0)), None,
                op0=Alu.is_ge,
            )
            nc.vector.tensor_scalar(
                bmask, mvals, float(np.float32(0.4) / np.float32(1.4)), None,
                op0=Alu.is_gt,
            )
            ba = stpool.tile([P, GTILES, B], fp32, tag="ba")
            nc.vector.tensor_sub(ba, bmask, amask)
            for bp, gsb in ((0, g01), (1, g23)):
                lab = gsb.rearrange("p t (j f) -> p t j f", j=2)[:, :, :, 8]
                la = stpool.tile([P, GTILES, 2], fp32, tag="la")
                nc.vector.tensor_mul(la, lab, amask[:, :, 2 * bp : 2 * bp + 2])
                nc.vector.tensor_sub(
                    cls_st[:, 2 * bp : 2 * bp + 2, gsl].transpose([0, 2, 1]),
                    la,
                    ba[:, :, 2 * bp : 2 * bp + 2],
                )

        # outputs in SBUF-natural order: DRAM[b, p*T + ti] (host unpermutes)
        for v in range(4):
            for b in range(B):
                nc.sync.dma_start(
                    out=loc_d[v][b].rearrange("(p t) -> p t", p=P),
                    in_=st[v][:, b, :],
                )
        for b in range(B):
            nc.sync.dma_start(
                out=cls_d[b].rearrange("(p t) -> p t", p=P),
                in_=cls_st[:, b, :],
            )

    nc.compile()
    return nc


# ----------------------------------------------------------------------------
# Host data preparation
# ----------------------------------------------------------------------------
def _hilo(v):
    import ml_dtypes

    hi = v.astype(ml_dtypes.bfloat16)
    lo = (v - hi.astype(np.float32)).astype(ml_dtypes.bfloat16)
    return hi.astype(np.float32), lo.astype(np.float32)


def _prepare_host(labels, boxes):
    import ml_dtypes

    f32 = np.float32
    bfl = ml_dtypes.bfloat16
    NX = sum(XTILES)

    wh_t = _anchor_wh()  # [4, NT, 2]
    aa_lvl = (wh_t[..., 0] + f32(1.0)) * (wh_t[..., 1] + f32(1.0))  # [4, NT]

    a_, b_ = boxes[..., :2].astype(f32), boxes[..., 2:].astype(f32)
    bxy = (a_ + b_) / f32(2.0)
    bwh = b_ - a_ + f32(1.0)
    b1 = np.concatenate([bxy - bwh / f32(2.0), bxy + bwh / f32(2.0)], -1)  # [B,N,4]
    area_b = (b1[..., 2] - b1[..., 0] + f32(1.0)) * (
        b1[..., 3] - b1[..., 1] + f32(1.0)
    )

    corners = np.empty((P, 2, B, N), f32)
    corners[:, 0] = b1[None, :, :, 0]  # bx1
    corners[:, 1] = b1[None, :, :, 2]  # bx2

    grids = []
    for i in range(4):
        fm = FMS[i]
        grid = S / fm
        grids.append(((np.arange(fm, dtype=f32) + f32(0.5)) * f32(grid)).astype(f32))

    # X-clamp constants per X-tile; pad rows force wx = 0
    xclamp = np.zeros((P, NX, 3), f32)
    xi = 0
    for lv in range(4):
        nb, gxw = NB[lv], GXW[lv]
        for tb in range(TBLK[lv]):
            p = np.arange(P)
            t = tb * nb + p // gxw
            gx = p % gxw
            valid = t < NT
            tcl = np.clip(t, 0, NT - 1)
            w = wh_t[lv, tcl, 0]
            cx = grids[lv][gx]
            ax1 = cx - w / f32(2.0)
            ax2 = cx + w / f32(2.0)
            wa1p = ax2 - ax1 + f32(1.0)
            xclamp[:, xi, 0] = np.where(valid, -ax1, f32(0.0))
            xclamp[:, xi, 1] = np.where(valid, ax2, f32(-1.0e6))
            xclamp[:, xi, 2] = np.where(valid, wa1p, f32(-1.0))
            xi += 1

    # onesel[base + i, lv, p] = 1 where p//gxw == i % nb, for i < 3*nb
    onesel = np.zeros((P, 4, P), f32)
    for base in (0, 32, 64):
        for lv in range(4):
            nb, gxw = NB[lv], GXW[lv]
            for i in range(3 * nb):
                onesel[base + i, lv, (i % nb) * gxw : (i % nb + 1) * gxw] = 1.0

    # gather tables (bf16 hi/lo): [P, 36]
    lw = np.log(bwh[..., 0])
    lh = np.log(bwh[..., 1])
    labf = labels.astype(f32)
    tabs = np.zeros((P, 36), f32)
    for bp in range(2):
        for j in range(2):
            b = 2 * bp + j
            rows = slice(j * N, (j + 1) * N)
            cols = []
            for v in (bxy[b, :, 0], bxy[b, :, 1], lw[b], lh[b]):
                hi, lo = _hilo(v)
                cols += [hi, lo]
            cols.append(labf[b])
            tabs[rows, bp * 18 + j * 9 : bp * 18 + j * 9 + 9] = np.stack(cols, -1)
    tables = tabs.astype(bfl)

    iota = np.broadcast_to(np.arange(N, dtype=f32)[None, :], (P, N)).copy()
    ident = np.eye(P, dtype=f32).astype(bfl)

    anchors = build_anchor_boxes(S)
    acx, acy, aw, ah = anchors[:, 0], anchors[:, 1], anchors[:, 2], anchors[:, 3]
    law, lah = np.log(aw), np.log(ah)
    invaw, invah = f32(1.0) / aw, f32(1.0) / ah

    percore = []
    for c in range(NC_COUNT):
        # wyrows = wy(gy,t) * invS(lv,t) per (b,n); [8, tile, 256] packing
        wyr = np.zeros((8, TILES_PER_CORE, FREE), f32)
        s = 0
        for lv in range(4):
            nb = NB[lv]
            cyv = grids[lv]
            for j in range(CORE_LEVEL_TILES[lv]):
                gtile = c * CORE_LEVEL_TILES[lv] + j
                gy = gtile // TBLK[lv]
                tblk = gtile % TBLK[lv]
                for k in range(nb):
                    t = tblk * nb + k
                    if t < NT:
                        h = wh_t[lv, t, 1]
                        cy = cyv[gy]
                        ay1 = cy - h / f32(2.0)
                        ay2 = cy + h / f32(2.0)
                        ha1p = ay2 - ay1 + f32(1.0)
                        rs = np.maximum(b1[..., 1] - ay1, f32(0.0))
                        rt = np.maximum(ay2 - b1[..., 3], f32(0.0))
                        wy = np.maximum(-((rs - ha1p) + rt), f32(0.0))
                        invs = f32(1.0) / (aa_lvl[lv, t] + area_b)
                        wyr[k, s] = (wy * invs).reshape(-1)
                s += 1
        # 3-term bf16 split packed per tile at partition base 32*(j%4)
        wyp = np.zeros((88, 67, FREE), f32)
        for j in range(TILES_PER_CORE):
            lv = next(
                i for i in range(4)
                if j < sum(CORE_LEVEL_TILES[: i + 1])
            )
            nb = NB[lv]
            base = 32 * (j % 3)
            col = j // 3
            for k in range(nb):
                w = wyr[k, j]
                hi = w.astype(bfl).astype(f32)
                mid = (w - hi).astype(bfl).astype(f32)
                lo = (w - hi - mid).astype(bfl).astype(f32)
                wyp[base + k, col] = hi
                wyp[base + nb + k, col] = mid
                wyp[base + 2 * nb + k, col] = lo
        wyrows = wyp.astype(bfl).reshape(88, 67 * FREE)

        gidx = _core_slot_anchor_index(c)  # [T, P], -1 = pad
        safe = np.where(gidx >= 0, gidx, 0)
        encc = np.stack(
            [acx[safe], acy[safe], invaw[safe], invah[safe], law[safe], lah[safe]],
            -1,
        ).astype(f32)
        encc = np.ascontiguousarray(encc.transpose(1, 0, 2)).reshape(
            P, TILES_PER_CORE * 6
        )

        percore.append(
            dict(
                corners=corners.reshape(P, 2 * FREE),
                xclamp=xclamp.reshape(P, NX * 3),
                encc=encc,
                wyrows=wyrows,
                onesel=onesel.astype(bfl).reshape(P, 4 * P),
                tables=tables,
                iota=iota,
                ident=ident,
            )
        )
    return percore


def _assemble(results, labels_dtype):
    cls_full = np.empty((B, A_TOTAL), np.float32)
    loc_full = np.empty((B, A_TOTAL, 4), np.float32)

    def unperm(a):
        # device emits [B, p*T + ti]; bring to slot-major [B, ti*P + p]
        return np.ascontiguousarray(
            a.reshape(B, P, TILES_PER_CORE).transpose(0, 2, 1).reshape(B, AC)
        )

    for c in range(NC_COUNT):
        r = results[c]
        gidx = _core_slot_anchor_index(c).reshape(-1)
        valid = gidx >= 0
        tgt = gidx[valid]
        cls_full[:, tgt] = unperm(r["clsf"])[:, valid]
        for v in range(4):
            loc_full[:, tgt, v] = unperm(r[f"loc{v}"])[:, valid]
    cls_out = cls_full.astype(labels_dtype)
    return cls_out, loc_full


def _fix_compiler_flags():
    """Skip the neuronxcc DataLocalityOpt pass (crashes on our DMA macros)."""
    from concourse import compiler_utils as cu

    flags = cu.get_compiler_flags()
    out = []
    for f in flags:
        if f.startswith("--tensorizer-options=") and "DataLocalityOpt" not in f:
            f = f.rstrip() + " --skip-pass=DataLocalityOpt "
        out.append(f)
    cu.set_compiler_flags(out)


def _run(labels, boxes, input_size, trace=False):
    from concourse.bass_utils import run_bass_kernel_spmd

    _fix_compiler_flags()

    assert int(input_size) == S, f"kernel hardcoded for input_size={S}"
    labels = np.asarray(labels)
    boxes = np.asarray(boxes, dtype=np.float32)

    if "prog" not in _prog_cache:
        _prog_cache["prog"] = _build_program()
    nc = _prog_cache["prog"]

    in_maps = _prepare_host(labels, boxes)
    res = run_bass_kernel_spmd(
        nc, in_maps, core_ids=list(range(NC_COUNT)), trace=trace
    )
    cls_out, loc_out = _assemble(res.results, labels.dtype)
    return (cls_out, loc_out), res


def kernel(labels, boxes, input_size):
    (cls_out, loc_out), _ = _run(labels, boxes, input_size)
    return cls_out, loc_out


# revision 33
# speedup vs baseline: 1.2115x; 1.0017x over previous
"""Anchor target-assignment (IoU match + encode) on 8 TRN2 NeuronCores.

Self-contained: hardcodes shapes for the nn_Anchors problem
(B=4, N=64, input_size=512, A=195840).

v2 design ("t-major" anchor reorder):
  Anchors are processed in (level, gy, t-block, gx) order so that each
  128-anchor tile has gx on partitions and a constant anchor-type block.
  - x-overlap factors wx(gx,t,b,n) live in 19 precomputed X-table tiles
    [128, 256] (computed on device from box corners, one relu pipeline per
    X-tile instead of per anchor-tile).
  - wy(gy,t,b,n) * invS(lv,t,b,n) is host-computed per tile as 1..8 rows
    [nb, 256]; a tiny PE ones-block matmul broadcasts it to [128, 256].
  - q = wx * (wy*invS) is a per-box monotone transform of IoU
    (q = inter/(area_a+area_b)); argmax_n q == argmax_n IoU and IoU
    thresholds 0.5/0.4 map to q thresholds 1/3 and 2/7.
  - argmax via tensor_reduce(max) + max_index (exact first-occurrence).
  - gather of matched (cx, cy, log w, log h, label) via bf16 one-hot
    PE transpose + matmul; coords/log-sizes are split hi+lo in bf16 to
    keep f32-level accuracy.
  Levels 1-3 pad their t-axis to tile multiples; padded X rows are zero
  (q=0) and the host drops pad slots when stitching.
"""

import math
import os
import sys

import numpy as np

for _p in ("/opt/trn_rl_repo", "/root/.axon_site/_ro/trn_rl_repo"):
    if os.path.isdir(_p) and _p not in sys.path:
        sys.path.insert(0, _p)

# ----------------------------------------------------------------------------
# Problem constants
# ----------------------------------------------------------------------------
B = 4
N = 64
S = 512
ANCHOR_AREAS = [4 * 4, 16 * 16, 64 * 64, 128 * 128]
ASPECT_RATIOS = [1 / 2.0, 1 / 1.0, 2 / 1.0]
SCALE_RATIOS = [1.0, 2.0, 3 / 4.0]
NT = 9  # anchor types per cell
FMS = [128, 64, 32, 16]
LEVEL_ANCHORS = [fm * fm * NT for fm in FMS]  # 147456, 36864, 9216, 2304
A_TOTAL = sum(LEVEL_ANCHORS)  # 195840
P = 128
NC_COUNT = 8
FREE = B * N  # 256

# t-blocking per level: tiles are [NB t-variants x (128/NB) gx cells]
NB = [1, 2, 4, 8]  # t rows per tile (level i)
GXW = [128, 64, 32, 16]  # gx cells per t row (= fm)
TBLK = [9, 5, 3, 2]  # t-blocks per gy row (t padded to NB*TBLK)
LEVEL_TILES = [FMS[i] * TBLK[i] for i in range(4)]  # 1152, 320, 96, 32
CORE_LEVEL_TILES = [t // NC_COUNT for t in LEVEL_TILES]  # 144, 40, 12, 4
TILES_PER_CORE = sum(CORE_LEVEL_TILES)  # 200
AC = TILES_PER_CORE * P  # 25600 slots per core
GROUPS = 8
GTILES = TILES_PER_CORE // GROUPS  # 25
XTILES = TBLK  # X-table tiles per level: 9, 5, 3, 2 -> 19
WYROWS = [CORE_LEVEL_TILES[i] * NB[i] for i in range(4)]  # 144, 80, 48, 32

_prog_cache = {}


# ----------------------------------------------------------------------------
# Host-side anchor construction (mirrors reference.py exactly)
# ----------------------------------------------------------------------------
def _anchor_wh():
    wh = []
    for s in ANCHOR_AREAS:
        for ar in ASPECT_RATIOS:
            h = math.sqrt(s / ar)
            w = ar * h
            for sr in SCALE_RATIOS:
                wh.append([w * sr, h * sr])
    return np.asarray(wh, np.float32).reshape(len(ANCHOR_AREAS), NT, 2)


def build_anchor_boxes(input_size):
    wh = _anchor_wh()
    out = []
    for i in range(len(ANCHOR_AREAS)):
        fm = int(math.ceil(input_size / 2.0 ** (i + 2)))
        grid = input_size / fm
        centers = (np.arange(fm, dtype=np.float32) + 0.5) * grid
        gx, gy = np.meshgrid(centers, centers)
        xy = np.stack([gx, gy], axis=-1)
        xy = np.broadcast_to(xy[:, :, None, :], (fm, fm, NT, 2))
        whl = np.broadcast_to(wh[i][None, None, :, :], (fm, fm, NT, 2))
        out.append(
            np.concatenate([xy, whl], axis=-1).astype(np.float32).reshape(-1, 4)
        )
    return np.concatenate(out, 0)


def _slot_meta():
    """Per-slot (level, xtile_index, wyrow_start) — identical on all cores."""
    meta = []
    for lv in range(4):
        for j in range(CORE_LEVEL_TILES[lv]):
            xt = sum(XTILES[:lv]) + (j % TBLK[lv])
            wyr0 = sum(WYROWS[:lv]) + j * NB[lv]
            meta.append((lv, xt, wyr0))
    return meta


def _core_slot_anchor_index(core):
    """[TILES_PER_CORE, P] global real-anchor index per slot, or -1 for pads."""
    lvl_base = np.cumsum([0] + LEVEL_ANCHORS)
    idx = np.full((TILES_PER_CORE, P), -1, np.int64)
    s = 0
    for lv in range(4):
        nb, gxw, tb = NB[lv], GXW[lv], TBLK[lv]
        for j in range(CORE_LEVEL_TILES[lv]):
            gtile = core * CORE_LEVEL_TILES[lv] + j
            gy = gtile // tb
            tblk = gtile % tb
            p = np.arange(P)
            t = tblk * nb + p // gxw
            gx = p % gxw
            a = lvl_base[lv] + (gy * gxw + gx) * NT + t
            idx[s] = np.where(t < NT, a, -1)
            s += 1
    return idx


# ----------------------------------------------------------------------------
# Bass program (one SPMD program, identical for all 8 cores)
# ----------------------------------------------------------------------------
def _build_program():
    from contextlib import ExitStack

    from concourse import bacc, mybir
    from concourse.tile import TileContext

    fp32 = mybir.dt.float32
    bf16 = mybir.dt.bfloat16
    u32 = mybir.dt.uint32
    Alu = mybir.AluOpType
    Act = mybir.ActivationFunctionType

    nc = bacc.Bacc(None, target_bir_lowering=False)

    NX = sum(XTILES)  # 19

    # inputs
    corners_d = nc.declare_dram_parameter("corners", [P, 2 * FREE], fp32, isOutput=False)
    xclamp_d = nc.declare_dram_parameter("xclamp", [P, NX * 3], fp32, isOutput=False)
    encc_d = nc.declare_dram_parameter("encc", [P, TILES_PER_CORE * 6], fp32, isOutput=False)
    wyr_d = nc.declare_dram_parameter("wyrows", [88, 67 * FREE], bf16, isOutput=False)
    ones_d = nc.declare_dram_parameter("onesel", [P, 4 * P], bf16, isOutput=False)
    tables_d = nc.declare_dram_parameter("tables", [P, 36], bf16, isOutput=False)
    iota_d = nc.declare_dram_parameter("iota", [P, N], fp32, isOutput=False)
    ident_d = nc.declare_dram_parameter("ident", [P, P], bf16, isOutput=False)
    # outputs (v-split loc + float cls), SBUF-natural order (host unpermutes)
    loc_d = [
        nc.declare_dram_parameter(f"loc{v}", [B, AC], fp32, isOutput=True)
        for v in range(4)
    ]
    cls_d = nc.declare_dram_parameter("clsf", [B, AC], fp32, isOutput=True)

    meta = _slot_meta()

    with TileContext(nc) as tc, ExitStack() as ctx:
        cpool = ctx.enter_context(tc.tile_pool(name="consts", bufs=1))
        corners = cpool.tile([P, 2, B, N], fp32)
        xclamp = cpool.tile([P, NX, 3], fp32)
        wyrows = cpool.tile([88, 67, FREE], bf16)
        onesel = cpool.tile([P, 4, P], bf16)
        tables = cpool.tile([P, 36], bf16)
        iota = cpool.tile([P, N], fp32)
        ident = cpool.tile([P, P], bf16)
        nc.sync.dma_start(out=corners, in_=corners_d[:, :])
        nc.sync.dma_start(out=xclamp, in_=xclamp_d[:, :])
        nc.sync.dma_start(out=wyrows, in_=wyr_d[:, :])
        nc.sync.dma_start(out=onesel, in_=ones_d[:, :])
        nc.sync.dma_start(out=tables, in_=tables_d[:, :])
        nc.sync.dma_start(out=iota, in_=iota_d[:, :])
        nc.sync.dma_start(out=ident, in_=ident_d[:, :])

        xpool = ctx.enter_context(tc.tile_pool(name="xtab", bufs=1))
        xprep = ctx.enter_context(tc.tile_pool(name="xprep", bufs=6))
        gpool = ctx.enter_context(tc.tile_pool(name="group", bufs=3))
        wpool = ctx.enter_context(tc.tile_pool(name="work", bufs=6))
        spool = ctx.enter_context(tc.tile_pool(name="small", bufs=6))
        ppool = ctx.enter_context(tc.tile_pool(name="psum", bufs=3, space="PSUM"))
        gapool = ctx.enter_context(tc.tile_pool(name="gacc", bufs=1, space="PSUM"))
        stpool = ctx.enter_context(tc.tile_pool(name="stage", bufs=2))
        opool = ctx.enter_context(tc.tile_pool(name="outstage", bufs=1))

        # ---------------- X-table prep: 19 tiles ----------------
        xtab = [
            xpool.tile([P, B, N], fp32, tag=f"x{i}", name=f"x{i}") for i in range(NX)
        ]
        for i in range(NX):
            nax1 = xclamp[:, i : i + 1, 0:1]
            ax2 = xclamp[:, i : i + 1, 1:2]
            wa1p = xclamp[:, i : i + 1, 2:3]
            rsx = xprep.tile([P, B, N], fp32, tag="rsx")
            rtx = xprep.tile([P, B, N], fp32, tag="rtx")
            nc.scalar.activation(rsx, corners[:, 0], Act.Relu, bias=nax1, scale=1.0)
            nc.scalar.activation(rtx, corners[:, 1], Act.Relu, bias=ax2, scale=-1.0)
            ssx = xprep.tile([P, B, N], fp32, tag="ssx")
            nc.vector.scalar_tensor_tensor(
                ssx, rsx, wa1p, rtx, op0=Alu.subtract, op1=Alu.add
            )
            nc.scalar.activation(xtab[i], ssx, Act.Relu, bias=0.0, scale=-1.0)

        st = [
            opool.tile([P, B, TILES_PER_CORE], fp32, tag=f"st{v}", name=f"st{v}")
            for v in range(4)
        ]
        cls_st = opool.tile([P, B, TILES_PER_CORE], fp32, tag="stc")

        for g in range(GROUPS):
            encc_g = gpool.tile([P, GTILES, 6], fp32, tag="encc")
            nc.sync.dma_start(
                out=encc_g, in_=encc_d[:, g * GTILES * 6 : (g + 1) * GTILES * 6]
            )
            m_acc = gpool.tile([P, GTILES, B, 8], fp32, tag="macc")
            nc.gpsimd.memset(m_acc, -1.0)

            g01_ps = gapool.tile([P, GTILES, 18], fp32, tag="g01")
            g23_ps = gapool.tile([P, GTILES, 18], fp32, tag="g23")

            for gi in range(GTILES):
                slot = g * GTILES + gi
                lv, xt, wyr0 = meta[slot]
                nb = NB[lv]

                # --- wyrep: 3-term bf16 broadcast matmul (sum is ~exact f32) ---
                wyrep_ps = ppool.tile([P, B, N], fp32, tag="wyrep")
                pb = 32 * (slot % 3)
                nc.tensor.matmul(
                    wyrep_ps,
                    onesel[pb : pb + 3 * nb, lv, :],
                    wyrows[pb : pb + 3 * nb, slot // 3, :],
                    start=True,
                    stop=True,
                )
                q = wpool.tile([P, B, N], fp32, tag="q")
                nc.vector.tensor_mul(q, xtab[xt], wyrep_ps)

                # --- max over boxes + argmax + one-hot ---
                nc.vector.tensor_reduce(
                    out=m_acc[:, gi : gi + 1, :, 0:1],
                    in_=q,
                    axis=mybir.AxisListType.X,
                    op=Alu.max,
                )
                i8 = spool.tile([P, B, 8], u32, tag="i8")
                i8f = spool.tile([P, B, 8], fp32, tag="i8f")
                for b in range(B):
                    nc.vector.max_index(i8[:, b, :], m_acc[:, gi, b, :], q[:, b, :])
                nc.scalar.copy(i8f, i8)
                # one-hot in a single broadcast TT
                onehot = wpool.tile([P, B, N], bf16, tag="onehot")
                nc.vector.tensor_tensor(
                    out=onehot,
                    in0=iota.unsqueeze(1).to_broadcast([P, B, N]),
                    in1=i8f[:, :, 0:1].to_broadcast([P, B, N]),
                    op=Alu.is_equal,
                )

                # --- gather via PE: transpose one-hot, matmul with tables ---
                ohT_ps = ppool.tile([P, 2, P], bf16, tag="ohT")
                oh2 = onehot.rearrange("p b n -> p (b n)")
                nc.tensor.transpose(ohT_ps[:, 0, :], oh2[:, 0:P], ident)
                nc.tensor.transpose(ohT_ps[:, 1, :], oh2[:, P : 2 * P], ident)
                ohT = spool.tile([P, 2, P], bf16, tag="ohTs")
                nc.scalar.copy(ohT, ohT_ps)
                nc.tensor.matmul(
                    g01_ps[:, gi, :], ohT[:, 0, :], tables[:, 0:18],
                    start=True, stop=True,
                )
                nc.tensor.matmul(
                    g23_ps[:, gi, :], ohT[:, 1, :], tables[:, 18:36],
                    start=True, stop=True,
                )

            # ---------------- per-group encode ----------------
            # gathered cols per (pair j): j*9 + [cxh,cxl,cyh,cyl,lwh,lwl,lhh,lhl,lab]
            g01 = stpool.tile([P, GTILES, 18], fp32, tag="g01s")
            g23 = stpool.tile([P, GTILES, 18], fp32, tag="g23s")
            nc.scalar.copy(g01, g01_ps)
            nc.scalar.copy(g23, g23_ps)

            gsl = slice(g * GTILES, (g + 1) * GTILES)
            enc = encc_g  # [P, GTILES, 6]: acx, acy, invaw, invah, lwa, lha

            for bp, gsb in ((0, g01), (1, g23)):
                gq = gsb.rearrange("p t (j f) -> p t j f", j=2)  # [P,GT,2,9]

                def outv(tile):
                    return tile[:, 2 * bp : 2 * bp + 2, gsl].transpose([0, 2, 1])

                # center coords: (hi + lo - a) * inv_size
                for v, si in ((0, 2), (1, 3)):
                    hl = stpool.tile([P, GTILES, 2], fp32, tag="enchl")
                    nc.vector.tensor_add(
                        hl, gq[:, :, :, 2 * v], gq[:, :, :, 2 * v + 1]
                    )
                    d = stpool.tile([P, GTILES, 2], fp32, tag="encd")
                    nc.vector.tensor_sub(
                        d, hl, enc[:, :, v : v + 1].to_broadcast([P, GTILES, 2])
                    )
                    nc.vector.tensor_mul(
                        outv(st[v]),
                        d,
                        enc[:, :, si : si + 1].to_broadcast([P, GTILES, 2]),
                    )
                # log-space wh: (lwh + lwl) - log(aw)
                for v, li in ((2, 4), (3, 5)):
                    hl = stpool.tile([P, GTILES, 2], fp32, tag="enchl")
                    nc.vector.tensor_add(
                        hl, gq[:, :, :, 2 * v], gq[:, :, :, 2 * v + 1]
                    )
                    nc.vector.tensor_sub(
                        outv(st[v]),
                        hl,
                        enc[:, :, li : li + 1].to_broadcast([P, GTILES, 2]),
                    )

            # cls: lab*a - (b2 - a);  a = [q >= 1/3], b2 = [q > 2/7]
            mvals = m_acc[:, :, :, 0]  # [P, GTILES, B] stride 8
            amask = stpool.tile([P, GTILES, B], fp32, tag="amask")
            bmask = stpool.tile([P, GTILES, B], fp32, tag="bmask")
            nc.vector.tensor_scalar(
                amask, mvals, float(np.float32(1.0) / np.float32(3.0)), None,
                op0=Alu.is_ge,
            )
            nc.vector.tensor_scalar(
                bmask, mvals, float(np.float32(0.4) / np.float32(1.4)), None,
                op0=Alu.is_gt,
            )
            ba = stpool.tile([P, GTILES, B], fp32, tag="ba")
            nc.vector.tensor_sub(ba, bmask, amask)
            for bp, gsb in ((0, g01), (1, g23)):
                lab = gsb.rearrange("p t (j f) -> p t j f", j=2)[:, :, :, 8]
                la = stpool.tile([P, GTILES, 2], fp32, tag="la")
                nc.vector.tensor_mul(la, lab, amask[:, :, 2 * bp : 2 * bp + 2])
                nc.vector.tensor_sub(
                    cls_st[:, 2 * bp : 2 * bp + 2, gsl].transpose([0, 2, 1]),
                    la,
                    ba[:, :, 2 * bp : 2 * bp + 2],
                )

            # stream outputs every 2 groups (SBUF-natural order, host unpermutes)
            if g % 2 == 1:
                osl = slice((g - 1) * GTILES, (g + 1) * GTILES)
                for v in range(4):
                    for b in range(B):
                        nc.sync.dma_start(
                            out=loc_d[v][b].rearrange("(p t) -> p t", p=P)[:, osl],
                            in_=st[v][:, b, osl],
                        )
                for b in range(B):
                    nc.sync.dma_start(
                        out=cls_d[b].rearrange("(p t) -> p t", p=P)[:, osl],
                        in_=cls_st[:, b, osl],
                    )

    nc.compile()
    return nc


# ----------------------------------------------------------------------------
# Host data preparation
# ----------------------------------------------------------------------------
def _hilo(v):
    import ml_dtypes

    hi = v.astype(ml_dtypes.bfloat16)
    lo = (v - hi.astype(np.float32)).astype(ml_dtypes.bfloat16)
    return hi.astype(np.float32), lo.astype(np.float32)


def _prepare_host(labels, boxes):
    import ml_dtypes

    f32 = np.float32
    bfl = ml_dtypes.bfloat16
    NX = sum(XTILES)

    wh_t = _anchor_wh()  # [4, NT, 2]
    aa_lvl = (wh_t[..., 0] + f32(1.0)) * (wh_t[..., 1] + f32(1.0))  # [4, NT]

    a_, b_ = boxes[..., :2].astype(f32), boxes[..., 2:].astype(f32)
    bxy = (a_ + b_) / f32(2.0)
    bwh = b_ - a_ + f32(1.0)
    b1 = np.concatenate([bxy - bwh / f32(2.0), bxy + bwh / f32(2.0)], -1)  # [B,N,4]
    area_b = (b1[..., 2] - b1[..., 0] + f32(1.0)) * (
        b1[..., 3] - b1[..., 1] + f32(1.0)
    )

    corners = np.empty((P, 2, B, N), f32)
    corners[:, 0] = b1[None, :, :, 0]  # bx1
    corners[:, 1] = b1[None, :, :, 2]  # bx2

    grids = []
    for i in range(4):
        fm = FMS[i]
        grid = S / fm
        grids.append(((np.arange(fm, dtype=f32) + f32(0.5)) * f32(grid)).astype(f32))

    # X-clamp constants per X-tile; pad rows force wx = 0
    xclamp = np.zeros((P, NX, 3), f32)
    xi = 0
    for lv in range(4):
        nb, gxw = NB[lv], GXW[lv]
        for tb in range(TBLK[lv]):
            p = np.arange(P)
            t = tb * nb + p // gxw
            gx = p % gxw
            valid = t < NT
            tcl = np.clip(t, 0, NT - 1)
            w = wh_t[lv, tcl, 0]
            cx = grids[lv][gx]
            ax1 = cx - w / f32(2.0)
            ax2 = cx + w / f32(2.0)
            wa1p = ax2 - ax1 + f32(1.0)
            xclamp[:, xi, 0] = np.where(valid, -ax1, f32(0.0))
            xclamp[:, xi, 1] = np.where(valid, ax2, f32(-1.0e6))
            xclamp[:, xi, 2] = np.where(valid, wa1p, f32(-1.0))
            xi += 1

    # onesel[base + i, lv, p] = 1 where p//gxw == i % nb, for i < 3*nb
    onesel = np.zeros((P, 4, P), f32)
    for base in (0, 32, 64):
        for lv in range(4):
            nb, gxw = NB[lv], GXW[lv]
            for i in range(3 * nb):
                onesel[base + i, lv, (i % nb) * gxw : (i % nb + 1) * gxw] = 1.0

    # gather tables (bf16 hi/lo): [P, 36]
    lw = np.log(bwh[..., 0])
    lh = np.log(bwh[..., 1])
    labf = labels.astype(f32)
    tabs = np.zeros((P, 36), f32)
    for bp in range(2):
        for j in range(2):
            b = 2 * bp + j
            rows = slice(j * N, (j + 1) * N)
            cols = []
            for v in (bxy[b, :, 0], bxy[b, :, 1], lw[b], lh[b]):
                hi, lo = _hilo(v)
                cols += [hi, lo]
            cols.append(labf[b])
            tabs[rows, bp * 18 + j * 9 : bp * 18 + j * 9 + 9] = np.stack(cols, -1)
    tables = tabs.astype(bfl)

    iota = np.broadcast_to(np.arange(N, dtype=f32)[None, :], (P, N)).copy()
    ident = np.eye(P, dtype=f32).astype(bfl)

    anchors = build_anchor_boxes(S)
    acx, acy, aw, ah = anchors[:, 0], anchors[:, 1], anchors[:, 2], anchors[:, 3]
    law, lah = np.log(aw), np.log(ah)
    invaw, invah = f32(1.0) / aw, f32(1.0) / ah

    percore = []
    for c in range(NC_COUNT):
        # wyrows = wy(gy,t) * invS(lv,t) per (b,n); [8, tile, 256] packing
        wyr = np.zeros((8, TILES_PER_CORE, FREE), f32)
        s = 0
        for lv in range(4):
            nb = NB[lv]
            cyv = grids[lv]
            for j in range(CORE_LEVEL_TILES[lv]):
                gtile = c * CORE_LEVEL_TILES[lv] + j
                gy = gtile // TBLK[lv]
                tblk = gtile % TBLK[lv]
                for k in range(nb):
                    t = tblk * nb + k
                    if t < NT:
                        h = wh_t[lv, t, 1]
                        cy = cyv[gy]
                        ay1 = cy - h / f32(2.0)
                        ay2 = cy + h / f32(2.0)
                        ha1p = ay2 - ay1 + f32(1.0)
                        rs = np.maximum(b1[..., 1] - ay1, f32(0.0))
                        rt = np.maximum(ay2 - b1[..., 3], f32(0.0))
                        wy = np.maximum(-((rs - ha1p) + rt), f32(0.0))
                        invs = f32(1.0) / (aa_lvl[lv, t] + area_b)
                        wyr[k, s] = (wy * invs).reshape(-1)
                s += 1
        # 3-term bf16 split packed per tile at partition base 32*(j%4)
        wyp = np.zeros((88, 67, FREE), f32)
        for j in range(TILES_PER_CORE):
            lv = next(
                i for i in range(4)
                if j < sum(CORE_LEVEL_TILES[: i + 1])
            )
            nb = NB[lv]
            base = 32 * (j % 3)
            col = j // 3
            for k in range(nb):
                w = wyr[k, j]
                hi = w.astype(bfl).astype(f32)
                mid = (w - hi).astype(bfl).astype(f32)
                lo = (w - hi - mid).astype(bfl).astype(f32)
                wyp[base + k, col] = hi
                wyp[base + nb + k, col] = mid
                wyp[base + 2 * nb + k, col] = lo
        wyrows = wyp.astype(bfl).reshape(88, 67 * FREE)

        gidx = _core_slot_anchor_index(c)  # [T, P], -1 = pad
        safe = np.where(gidx >= 0, gidx, 0)
        encc = np.stack(
            [acx[safe], acy[safe], invaw[safe], invah[safe], law[safe], lah[safe]],
            -1,
        ).astype(f32)
        encc = np.ascontiguousarray(encc.transpose(1, 0, 2)).reshape(
            P, TILES_PER_CORE * 6
        )

        percore.append(
            dict(
                corners=corners.reshape(P, 2 * FREE),
                xclamp=xclamp.reshape(P, NX * 3),
                encc=encc,
                wyrows=wyrows,
                onesel=onesel.astype(bfl).reshape(P, 4 * P),
                tables=tables,
                iota=iota,
                ident=ident,
            )
        )
    return percore


def _assemble(results, labels_dtype):
    cls_full = np.empty((B, A_TOTAL), np.float32)
    loc_full = np.empty((B, A_TOTAL, 4), np.float32)

    def unperm(a):
        # device emits [B, p*T + ti]; bring to slot-major [B, ti*P + p]
        return np.ascontiguousarray(
            a.reshape(B, P, TILES_PER_CORE).transpose(0, 2, 1).reshape(B, AC)
        )

    for c in range(NC_COUNT):
        r = results[c]
        gidx = _core_slot_anchor_index(c).reshape(-1)
        valid = gidx >= 0
        tgt = gidx[valid]
        cls_full[:, tgt] = unperm(r["clsf"])[:, valid]
        for v in range(4):
            loc_full[:, tgt, v] = unperm(r[f"loc{v}"])[:, valid]
    cls_out = cls_full.astype(labels_dtype)
    return cls_out, loc_full


def _fix_compiler_flags():
    """Skip the neuronxcc DataLocalityOpt pass (crashes on our DMA macros)."""
    from concourse import compiler_utils as cu

    flags = cu.get_compiler_flags()
    out = []
    for f in flags:
        if f.startswith("--tensorizer-options=") and "DataLocalityOpt" not in f:
            f = f.rstrip() + " --skip-pass=DataLocalityOpt "
        out.append(f)
    cu.set_compiler_flags(out)


def _run(labels, boxes, input_size, trace=False):
    from concourse.bass_utils import run_bass_kernel_spmd

    _fix_compiler_flags()

    assert int(input_size) == S, f"kernel hardcoded for input_size={S}"
    labels = np.asarray(labels)
    boxes = np.asarray(boxes, dtype=np.float32)

    if "prog" not in _prog_cache:
        _prog_cache["prog"] = _build_program()
    nc = _prog_cache["prog"]

    in_maps = _prepare_host(labels, boxes)
    res = run_bass_kernel_spmd(
        nc, in_maps, core_ids=list(range(NC_COUNT)), trace=trace
    )
    cls_out, loc_out = _assemble(res.results, labels.dtype)
    return (cls_out, loc_out), res


def kernel(labels, boxes, input_size):
    (cls_out, loc_out), _ = _run(labels, boxes, input_size)
    return cls_out, loc_out
